# revision 35
# baseline (speedup 1.0000x reference)
"""Trainium2 Bass kernel for a 2-layer GCN link predictor (NetLinkTrain).

Math: z = relu(A @ (x @ W1)); z2 = A @ (z @ W2); out = [z2[e0], z2[e1]] @ Wlin.T
where A = D^-1/2 (Adj + I) D^-1/2.

Key algebraic factorizations:
  * W2/Wlin folding: W2' = W2 @ [Wlin[:, :128].T | Wlin[:, 128:].T] ([128, 4]),
    so layer 2 and the decode work on 4-wide node features.
  * The symmetric norm dinv[src]*dinv[dst] factors entirely out of both
    aggregations: gather from xd = dinv*x, then
      z = dinv[dst] * relu(W1^T @ sum xd[src])        (relu commutes, dinv>0)
      y = dinv[dst]^2 * (z_hat @ W2')                 (folds both dst factors)
      c = dinv[dst] * sum y[src]
    so the scatter one-hots are pure (iota == dst) compares with no weights.

Data movement (the memory-bound core of the problem):
  * L1 gathers 256B bf16 xd rows per edge (dma_gather, 1 desc/edge).
  * L2/decode gather ONLY the 4 bf16 values needed per edge (8B elements at a
    256B-aligned element stride from a padded table, 2 nodes per 256B row).
    Sub-256B elements need a patched dma_gather builder (the stock bass
    assert requires elem_size%256B; the hardware only requires the element
    STRIDE to be a 256B multiple -- verified on device).
  * Aggregation per 128-dst tile: TensorE scatter matmul with a DVE-built
    one-hot, accumulating in PSUM.

Sharding: edges are sharded by destination-node range (core c owns nodes
[c*6250, (c+1)*6250)); each core owns its segment sums completely, so the only
collectives are two small AllGathers (y and c, ~50-100KB per core).

Host (numpy) does index preprocessing only: self-loops, degrees, sorting edges
by (dst tile, table class), wrapped int16 index layout, and the xd cast.
"""

import math
import os
import sys

import numpy as np

sys.path.insert(0, "/opt/trn_rl_repo")

import concourse.bacc as bacc
import concourse.bass as bass
import concourse.tile as tile
from concourse import ap_utils, mybir
from concourse.bass_utils import run_bass_kernel_spmd

N = 50000
H = 128
P = 128
NC = 8
NPC = N // NC              # 6250 nodes per core
NT = math.ceil(NPC / P)    # 49 dst tiles per core
PADN = NT * P              # 6272 padded nodes per core
NSLOT = NC * PADN          # 50176 packed slots
LO = 25000                 # x-table half split (int16 index limit)
G1 = 7                     # L1 tiles per gather call group (49 = 7*7)
G2 = 13                    # L2 tiles per gather call group
NE_EVAL = 200000
EV_PC = NE_EVAL // NC      # 25000 eval edges per core

F32 = mybir.dt.float32
BF16 = mybir.dt.bfloat16
I16 = mybir.dt.int16
I8 = mybir.dt.int8

import ml_dtypes

BF16_NP = ml_dtypes.bfloat16


def _dma_gather_small(g, out_ap, in_ap, idxs_ap, num_idxs, elem_size, elem_step,
                      queue_num=0):
    """bass.BassGpSimd.dma_gather (DRAM source, non-transpose) without the
    elem_size%256B assert. The element STRIDE (elem_step bytes) must still be
    a 256B multiple; sub-256B elem_size verified on hardware."""
    g._assert_queue_num(queue_num)
    assert idxs_ap.dtype == mybir.dt.int16
    assert in_ap.dtype == out_ap.dtype
    assert ap_utils.ap_is_contiguous(in_ap.ap[1:])
    assert ap_utils.ap_is_contiguous(out_ap.ap[1:])
    assert ap_utils.ap_is_contiguous(idxs_ap.ap[1:])
    assert in_ap.ap[-1][1] == out_ap.ap[-1][1] == elem_size
    assert in_ap.ap[0][0] == elem_step
    stride_bytes = elem_step * mybir.dt.size(in_ap.dtype)
    stride_bytes_256, rem = divmod(stride_bytes, 256)
    assert rem == 0 and stride_bytes_256 < 256
    _in_ap = g.lower_ap_dma(in_ap, for_custom_bir_dma=True)
    return g.add_instruction(
        mybir.InstDMAGatherAnt(
            name=g.bass.get_next_instruction_name(),
            ins=[*_in_ap, g.lower_ap(idxs_ap),
                 g.lower_val_access(g.to_reg(num_idxs))],
            outs=[g.lower_ap(out_ap)],
            transpose=False,
            num_idxs=num_idxs,
            elem_size=elem_size,
            stride_bytes_256=stride_bytes_256,
            gen_mode=0,
            single_packet=False,
            queue_num=queue_num,
            sbuf_tokens_per_rank=0,
            sbuf_free_dim_per_rank=0,
            sbuf_free_dim_pad_per_rank=0,
            sbuf_byte_offset=0,
        )
    )


def _packed_id(n):
    """Packed slot of node n in the AllGathered y/c buffers: core-major,
    then natural node order (tile-major)."""
    n = np.asarray(n)
    c = n // NPC
    return c * PADN + (n - c * NPC)


def _wrap_idx(v, n_chunks):
    """v: [n_chunks, 128] int -> dma_gather wrapped idx layout [128, n_chunks*8]."""
    a16 = v.reshape(n_chunks, 8, 16).transpose(2, 0, 1).reshape(16, n_chunks * 8)
    return np.tile(a16, (8, 1)).astype(np.int16)


def _edge_layout(core, tl, cls, dloc, idxval, group_sz, sval=None, ncls=2):
    """Lay out edges into (tile, class)-grouped 128-slot chunks.

    Gather-call contiguity: chunks ordered group-major; within a group all
    class-k chunks (tile-major) for k = 0..ncls-1.

    Returns (idx_wrapped, dstl[, scale]) per core plus shared shape info.
    """
    key = (core * NT + tl) * ncls + cls
    counts = np.zeros((NC, NT, ncls), np.int64)
    np.add.at(counts, (core, tl, cls), 1)
    nch_tc = -(-counts.max(axis=0) // P)  # [NT, ncls] chunks per (tile, class)

    ngrp = math.ceil(NT / group_sz)
    chunk_start = np.zeros((NT, ncls), np.int64)
    grp_runs = [[] for _ in range(ngrp)]  # per group: [(base, count)] per class
    pos = 0
    for g in range(ngrp):
        t0, t1 = g * group_sz, min((g + 1) * group_sz, NT)
        for k in range(ncls):
            base = pos
            for t in range(t0, t1):
                chunk_start[t, k] = pos
                pos += nch_tc[t, k]
            grp_runs[g].append((base, pos - base))
    grp_lo = [r[0] for r in grp_runs]
    grp_hi = [r[1] for r in grp_runs]
    nch = pos

    order = np.argsort(key, kind="stable")
    s_key = key[order]
    group_start = np.concatenate([[0], np.cumsum(counts.reshape(-1))])[:-1]
    rank = np.arange(len(order)) - group_start[s_key]
    s_core = s_key // (NT * ncls)
    s_t = (s_key // ncls) % NT
    s_cls = s_key % ncls
    dest = chunk_start[s_t, s_cls] * P + rank

    per_core = []
    for c in range(NC):
        m = s_core == c
        slot_idx = np.zeros(nch * P, np.int64)
        slot_dstl = np.full(nch * P, 255.0, np.float32)
        d = dest[m]
        slot_idx[d] = idxval[order][m]
        slot_dstl[d] = dloc[order][m]
        ent = [_wrap_idx(slot_idx.reshape(nch, P), nch),
               slot_dstl.reshape(nch, P).T.copy()]
        if sval is not None:
            slot_s = np.zeros(nch * P, np.float32)
            slot_s[d] = sval[order][m]
            ent.append(slot_s.reshape(nch, P).T.copy())
        per_core.append(tuple(ent))
    shape = dict(nch=nch, nch_tc=nch_tc.tolist(), chunk_start=chunk_start.tolist(),
                 grp_lo=grp_lo, grp_hi=grp_hi, grp_runs=grp_runs)
    return per_core, shape


def _preprocess(x, edge_index, pos_edge_index, neg_edge_index):
    x = np.asarray(x, np.float32)
    ei = np.asarray(edge_index)
    src = np.concatenate([ei[0], np.arange(N)]).astype(np.int64)
    dst = np.concatenate([ei[1], np.arange(N)]).astype(np.int64)
    deg = np.bincount(dst, minlength=N).astype(np.float64)
    dinv = 1.0 / np.sqrt(deg)  # every node has a self loop -> deg >= 1

    core = dst // NPC
    dl = dst - core * NPC
    tl = dl // P
    dloc = (dl % P).astype(np.float32)

    # L1: class = src parity (even/odd 256B rows of the int8 x table);
    # per-slot int8 dequant scale rides in the one-hot's second op
    xd = x.astype(np.float32) * dinv[:, None].astype(np.float32)
    s_node = (np.abs(xd).max(axis=1) / 127.0).astype(np.float32)
    l1, shape1 = _edge_layout(core, tl, src % 2, dloc, src // 2, G1,
                              sval=s_node[src])
    # L2: 4 classes = (src tile-half, slot parity); half-A sources (local
    # tiles 0-24) live in a separate padded table that is complete mid-L1,
    # so half-A aggregation overlaps the end of layer 1. The appended
    # self-loops are excluded -- their contribution is added from the local
    # y shard on-device (exact, no gather).
    ne_real = ei.shape[1]
    score = src // NPC
    soff = src - score * NPC
    in_b = (soff >= 3200).astype(np.int64)
    row = np.where(in_b == 0, score * 1600 + soff // 2,
                   score * 1536 + (soff - 3200) // 2)
    cls4 = in_b * 2 + soff % 2
    l2, shape2 = _edge_layout(core[:ne_real], tl[:ne_real], cls4[:ne_real],
                              dloc[:ne_real], row[:ne_real], G2, ncls=4)

    # per-(p, t) dinv and dinv^2 for the post-aggregation scales
    offs = np.arange(NT * P).reshape(NT, P)  # off = t*128 + p
    dinv_t = np.zeros((NC, P, NT), np.float32)
    dinv2_t = np.zeros((NC, P, NT), np.float32)
    for c in range(NC):
        v = np.zeros(NT * P)
        vv = dinv[c * NPC:(c + 1) * NPC]
        v[:NPC] = vv
        dinv_t[c] = v[offs].T
        v[:NPC] = vv * vv
        dinv2_t[c] = v[offs].T

    # decode: bucket eval edges by (slot(e0)%2, slot(e1)%2)
    e0 = np.concatenate([np.asarray(pos_edge_index[0]), np.asarray(neg_edge_index[0])])
    e1 = np.concatenate([np.asarray(pos_edge_index[1]), np.asarray(neg_edge_index[1])])
    s0, s1 = _packed_id(e0), _packed_id(e1)
    bkt = (s0 % 2) * 2 + (s1 % 2)
    bcnt = np.zeros((NC, 4), np.int64)
    for c in range(NC):
        bcnt[c] = np.bincount(bkt[c * EV_PC:(c + 1) * EV_PC], minlength=4)
    dcb = (-(-bcnt.max(axis=0) // P)).tolist()  # chunks per bucket (shared)
    dbase = np.concatenate([[0], np.cumsum(dcb)]).tolist()
    dc2 = int(dbase[-1])

    dec = []
    for c in range(NC):
        sl = slice(c * EV_PC, (c + 1) * EV_PC)
        b = bkt[sl]
        order = np.argsort(b, kind="stable")
        rank = np.arange(EV_PC) - np.concatenate([[0], np.cumsum(bcnt[c])])[:-1][b[order]]
        slot = np.array(dbase)[b[order]] * P + rank
        i0 = np.zeros(dc2 * P, np.int64)
        i1 = np.zeros(dc2 * P, np.int64)
        i0[slot] = (s0[sl] // 2)[order]
        i1[slot] = (s1[sl] // 2)[order]
        dec.append(dict(
            d_idx0=_wrap_idx(i0.reshape(dc2, P), dc2),
            d_idx1=_wrap_idx(i1.reshape(dc2, P), dc2),
            # slot -> position in this core's eval range
            unperm=(np.asarray(order), slot),
        ))

    per_core = []
    for c in range(NC):
        per_core.append(dict(
            idx1=l1[c][0], dstl1=l1[c][1], scl1=l1[c][2],
            idx2=l2[c][0], dstl2=l2[c][1],
            dinv_t=dinv_t[c], dinv2_t=dinv2_t[c],
            d_idx0=dec[c]["d_idx0"], d_idx1=dec[c]["d_idx1"],
            unperm=dec[c]["unperm"],
        ))
    shape = dict(l1=shape1, l2=shape2, dcb=dcb, dbase=dbase, dc2=dc2)
    return per_core, shape


def _build_program(shape):
    s1, s2 = shape["l1"], shape["l2"]
    NCH1, NCH2, DC2 = s1["nch"], s2["nch"], shape["dc2"]
    dbase, dcb = shape["dbase"], shape["dcb"]
    mx1 = max(max(r) for r in s1["nch_tc"])
    mx2 = max(max(r) for r in s2["nch_tc"])
    shape_l1_grp = [
        (s1["grp_lo"][g][0], s1["grp_hi"][g][0] + s1["grp_hi"][g][1])
        for g in range(len(s1["grp_lo"]))
    ]

    nc = bacc.Bacc("TRN2", target_bir_lowering=False, debug=False, num_devices=NC)

    xq_ap = nc.dram_tensor("xq", [N, 256], I8, kind="ExternalInput").ap()
    w1_ap = nc.dram_tensor("w1", [H, H], F32, kind="ExternalInput").ap()
    w2p_ap = nc.dram_tensor("w2p", [H, 4], F32, kind="ExternalInput").ap()
    # metadata blobs: one int16, one f32
    IW1 = NCH1 * 8
    IW2 = (NCH2 + 2 * DC2) * 8
    FW = 2 * NCH1 + NCH2 + 2 * NT
    iblob1_ap = nc.dram_tensor("iblob1", [P, IW1], I16, kind="ExternalInput").ap()
    iblob2_ap = nc.dram_tensor("iblob2", [P, IW2], I16, kind="ExternalInput").ap()
    fblob_ap = nc.dram_tensor("fblob", [P, FW], F32, kind="ExternalInput").ap()
    out_ap = nc.dram_tensor("out", [P, DC2 * 2], F32, kind="ExternalOutput").ap()

    phases = int(os.environ.get("K_PHASES", "3"))

    with tile.TileContext(nc) as tc:
        with (
            tc.tile_pool(name="persist", bufs=1) as pp,
            tc.tile_pool(name="g1", bufs=4) as g1p,
            tc.tile_pool(name="g2", bufs=3) as g2p,
            tc.tile_pool(name="oh", bufs=12) as ohp,
            tc.tile_pool(name="oh2", bufs=16) as ohp2,
            tc.tile_pool(name="small", bufs=8) as sp,
            tc.tile_pool(name="psA", bufs=4, space="PSUM") as psA,
            tc.tile_pool(name="psB", bufs=2, space="PSUM") as psB,
            tc.tile_pool(name="psC", bufs=2, space="PSUM") as psC,
            tc.tile_pool(name="dram", bufs=1, space="DRAM") as dp,
        ):
            # ---- persistent metadata ----
            iblob1 = pp.tile([P, IW1], I16)
            iblob2 = pp.tile([P, IW2], I16)
            fblob = pp.tile([P, FW], F32)
            for _g in range(math.ceil(NT / G1)):
                _lo = shape_l1_grp[_g][0] * 8
                _hi = shape_l1_grp[_g][1] * 8
                nc.sync.dma_start(out=iblob1[:, _lo:_hi], in_=iblob1_ap[:, _lo:_hi])
                if _g == 0:
                    nc.sync.dma_start(out=fblob[:], in_=fblob_ap[:])
            idx1_sb = iblob1[:]
            idx2_sb = iblob2[:, 0:NCH2 * 8]
            di0_sb = iblob2[:, NCH2 * 8:(NCH2 + DC2) * 8]
            di1_sb = iblob2[:, (NCH2 + DC2) * 8:]
            dstl1_sb = fblob[:, 0:NCH1]
            scl1_sb = fblob[:, NCH1:2 * NCH1]
            dstl2_sb = fblob[:, 2 * NCH1:2 * NCH1 + NCH2]
            dinv_sb = fblob[:, 2 * NCH1 + NCH2:2 * NCH1 + NCH2 + NT]
            dinv2_sb = fblob[:, 2 * NCH1 + NCH2 + NT:]

            w1f = pp.tile([H, H], F32)
            w2pf = pp.tile([H, 4], F32)
            nc.sync.dma_start(out=w1f[:], in_=w1_ap[:])
            nc.sync.dma_start(out=w2pf[:], in_=w2p_ap[:])
            w1b = pp.tile([H, H], BF16)
            w2pb = pp.tile([H, 4], BF16)
            nc.vector.tensor_copy(out=w1b[:], in_=w1f[:])
            nc.vector.tensor_copy(out=w2pb[:], in_=w2pf[:])

            iota_f = pp.tile([P, P], BF16)
            nc.gpsimd.iota(iota_f[:], pattern=[[1, P]], base=0,
                           channel_multiplier=0,
                           allow_small_or_imprecise_dtypes=True)

            y_sb = pp.tile([P, NT * 4], BF16)
            ydinv_sb = pp.tile([P, NT * 4], BF16)
            partA_sb = pp.tile([P, NT * 4], F32)
            qd_sb = pp.tile([P, NT * 4], F32)
            c_sb = pp.tile([P, NT * 4], BF16)
            out_sb = pp.tile([P, DC2 * 2], F32)

            SPLITS = [0, 13, 25, 37, NT]
            y_shard = dp.tile([PADN, 4], BF16)
            y_padA = dp.tile([NC * 1600, 128], BF16)
            y_padB = dp.tile([NC * 1536, 128], BF16)
            c_shard = dp.tile([PADN, 4], BF16)
            c_pad = dp.tile([NSLOT // 2, 128], BF16)
            y_fulls, c_fulls = [], []
            for k in range(4):
                nk = NC * (SPLITS[k + 1] - SPLITS[k]) * P
                y_fulls.append(dp.tile([nk, 4], BF16, name=f"y_full{k}"))
                c_fulls.append(dp.tile([nk, 4], BF16, name=f"c_full{k}"))

            xq2 = xq_ap[:].rearrange("(a b) e -> a b e", b=2)
            x_lo = xq2[:, 0, 0:H]
            x_hi = xq2[:, 1, 0:H]

            def build_oh(dstl_sb, c0, cnt, scl_sb=None, pool=None, mx=16,
                         act_nd=None):
                oh = (pool or ohp).tile([P, mx * P], BF16, tag="oh")
                for j in range(cnt):
                    if act_nd is not None and j % 4 == 0:
                        # ACT two-pass: a = |iota - d|; oh = relu(1 - a)
                        a = sp.tile([P, P], BF16, tag="oha")
                        nc.scalar.activation(
                            out=a[:], in_=iota_f[:],
                            func=mybir.ActivationFunctionType.Abs,
                            bias=act_nd[:, c0 + j:c0 + j + 1])
                        nc.scalar.activation(
                            out=oh[:, j * P:(j + 1) * P], in_=a[:],
                            func=mybir.ActivationFunctionType.Relu,
                            bias=1.0, scale=-1.0)
                        continue
                    nc.vector.tensor_scalar(
                        out=oh[:, j * P:(j + 1) * P],
                        in0=iota_f[:],
                        scalar1=dstl_sb[:, c0 + j:c0 + j + 1],
                        scalar2=(scl_sb[:, c0 + j:c0 + j + 1]
                                 if scl_sb is not None else None),
                        op0=mybir.AluOpType.is_equal,
                        op1=(mybir.AluOpType.mult
                             if scl_sb is not None else mybir.AluOpType.bypass),
                    )
                return oh[:, :cnt * P].rearrange("p (c e) -> p c e", e=P)

            # ---------------- Layer 1 + overlapped L2 pass-A ----------------
            Y_OFF_A = [0, 13 * 64]    # repad row offsets in y_padA (tiles 0-12, 13-24)
            Y_OFF_B = [0, 12 * 64]    # in y_padB (tiles 25-36, 37-48)

            def emit_y_split(k):
                t0, t1 = SPLITS[k], SPLITS[k + 1]
                nt = t1 - t0
                nc.sync.dma_start(
                    out=y_shard[t0 * P:t1 * P, :].rearrange(
                        "(t p) f -> p t f", p=P),
                    in_=y_sb[:, t0 * 4:t1 * 4].rearrange("p (t f) -> p t f", f=4),
                )
                if os.environ.get("K_NOCC"):
                    nc.sync.dma_start(out=y_fulls[k][0:P, :],
                                      in_=y_sb[:, t1 * 4 - 4:t1 * 4])
                else:
                    nc.gpsimd.collective_compute(
                        "AllGather", mybir.AluOpType.bypass,
                        replica_groups=[list(range(NC))],
                        ins=[y_shard[t0 * P:t1 * P, :].opt()],
                        outs=[y_fulls[k][:].opt()],
                    )
                pad, off = (y_padA, Y_OFF_A[k]) if k < 2 else (y_padB, Y_OFF_B[k - 2])
                nc.sync.dma_start(
                    out=pad[:, 0:8].rearrange(
                        "(c r) w -> c r w", c=NC)[:, off:off + nt * 64, :],
                    in_=y_fulls[k][:].rearrange(
                        "(c r q) f -> c r (q f)", c=NC, q=2),
                )

            gath2 = pp.tile([P, NCH2 * 4], BF16)
            g3d2 = gath2[:].rearrange("p (c e) -> p c e", e=4)

            def emit_l2_gathers(classes):
                for g in range(math.ceil(NT / G2)):
                    for k in classes:
                        base, cnt = s2["grp_runs"][g][k]
                        if cnt == 0:
                            continue
                        pad = y_padA if k < 2 else y_padB
                        _dma_gather_small(
                            nc.gpsimd,
                            out_ap=g3d2[:, base:base + cnt, :],
                            in_ap=pad[:, (k % 2) * 4:(k % 2) * 4 + 4],
                            idxs_ap=idx2_sb[:, base * 8:(base + cnt) * 8],
                            num_idxs=cnt * P,
                            elem_size=4,
                            elem_step=128,
                        )

            def emit_l2_tiles(t0, t1, classes, fold):
                for t in range(t0, t1):
                    runs = [(s2["chunk_start"][t][k], s2["nch_tc"][t][k])
                            for k in classes]
                    cnt = sum(cn for _, cn in runs)
                    if classes == (0, 1) and t in ohA_pre:
                        ohs = ohA_pre.pop(t)
                    else:
                        ohs = [build_oh(dstl2_sb, cb, cn, pool=ohp2, mx=mx2)
                               if cn else None for cb, cn in runs]
                    c_ps = psC.tile([P, 4], F32, tag="p4")
                    kk = 0
                    for (cb, cn), o3 in zip(runs, ohs):
                        for j in range(cn):
                            nc.tensor.matmul(
                                out=c_ps[:], lhsT=o3[:, j, :],
                                rhs=g3d2[:, cb + j, :],
                                start=(kk == 0), stop=(kk == cnt - 1),
                            )
                            kk += 1
                    fold(t, c_ps)

            def fold_a(t, c_ps):
                nc.scalar.copy(out=partA_sb[:, t * 4:t * 4 + 4], in_=c_ps[:])

            def fold_b(t, c_ps):
                nc.vector.scalar_tensor_tensor(
                    out=c_sb[:, t * 4:t * 4 + 4], in0=c_ps[:],
                    scalar=dinv_sb[:, t:t + 1],
                    in1=qd_sb[:, t * 4:t * 4 + 4],
                    op0=mybir.AluOpType.mult,
                    op1=mybir.AluOpType.add,
                )

            ohA_pre = {}

            def prebuild_passA(t0, t1):
                for t in range(t0, t1):
                    runs = [(s2["chunk_start"][t][k], s2["nch_tc"][t][k])
                            for k in (0, 1)]
                    ohA_pre[t] = [build_oh(dstl2_sb, cb, cn, pool=ohp2, mx=mx2)
                                  if cn else None for cb, cn in runs]

            ngrp1 = math.ceil(NT / G1)
            g1max = max(s1["grp_lo"][g][1] + s1["grp_hi"][g][1] for g in range(ngrp1))
            for g in range(ngrp1):
                if g == ngrp1 - 1:
                    nc.sync.dma_start(out=iblob2[:], in_=iblob2_ap[:])
                if phases >= 2 and g == 5:
                    prebuild_passA(0, 3)
                if phases >= 2 and g == 6:
                    prebuild_passA(3, 6)
                lo_b, lo_n = s1["grp_lo"][g]
                hi_b, hi_n = s1["grp_hi"][g]
                gcnt = lo_n + hi_n
                gq = g1p.tile([P, g1max * H], I8, tag="g1q")
                q3d = gq[:, :gcnt * H].rearrange("p (c e) -> p c e", e=H)
                for base, cnt, table in ((lo_b, lo_n, x_lo), (hi_b, hi_n, x_hi)):
                    if cnt == 0:
                        continue
                    _dma_gather_small(
                        nc.gpsimd,
                        out_ap=q3d[:, base - lo_b:base - lo_b + cnt, :],
                        in_ap=table,
                        idxs_ap=idx1_sb[:, base * 8:(base + cnt) * 8],
                        num_idxs=cnt * P,
                        elem_size=H,
                        elem_step=512,
                    )

                for t in range(g * G1, min((g + 1) * G1, NT)):
                    cl0, cn0 = s1["chunk_start"][t][0], s1["nch_tc"][t][0]
                    ch0, cn1 = s1["chunk_start"][t][1], s1["nch_tc"][t][1]
                    cnt = cn0 + cn1
                    msgs = sp.tile([P, 16 * H], BF16, tag="msgs")
                    nc.scalar.copy(out=msgs[:, :cn0 * H],
                                   in_=q3d[:, cl0 - lo_b:cl0 - lo_b + cn0, :])
                    if cn1:
                        nc.scalar.copy(out=msgs[:, cn0 * H:cnt * H],
                                       in_=q3d[:, ch0 - lo_b:ch0 - lo_b + cn1, :])
                    m3d = msgs[:, :cnt * H].rearrange("p (c e) -> p c e", e=H)
                    oh = build_oh(dstl1_sb, cl0, cn0, scl1_sb, mx=mx1)
                    oh2 = (build_oh(dstl1_sb, ch0, cn1, scl1_sb, mx=mx1)
                           if cn1 else None)
                    ut_ps = psA.tile([P, P], F32, tag="ut")
                    k = 0
                    for (cb, cn, o3) in ((cl0, cn0, oh), (ch0, cn1, oh2)):
                        for j in range(cn):
                            nc.tensor.matmul(
                                out=ut_ps[:], lhsT=m3d[:, k, :], rhs=o3[:, j, :],
                                start=(k == 0), stop=(k == cnt - 1),
                            )
                            k += 1
                    ut_sb = sp.tile([P, P], BF16, tag="utsb")
                    nc.scalar.copy(out=ut_sb[:], in_=ut_ps[:])
                    vt_ps = psB.tile([P, P], F32, tag="vt")
                    nc.tensor.matmul(out=vt_ps[:], lhsT=w1b[:], rhs=ut_sb[:],
                                     start=True, stop=True)
                    zt_sb = sp.tile([P, P], BF16, tag="ztsb")
                    nc.vector.tensor_relu(out=zt_sb[:], in_=vt_ps[:])
                    y_ps = psC.tile([P, 4], F32, tag="p4")
                    nc.tensor.matmul(out=y_ps[:], lhsT=zt_sb[:], rhs=w2pb[:],
                                     start=True, stop=True)
                    nc.vector.tensor_scalar(
                        out=y_sb[:, t * 4:t * 4 + 4], in0=y_ps[:],
                        scalar1=dinv2_sb[:, t:t + 1], scalar2=None,
                        op0=mybir.AluOpType.mult,
                    )

            for k in range(4):
                emit_y_split(k)
            if phases >= 2:
                emit_l2_gathers((0, 1))
                emit_l2_tiles(0, NT, (0, 1), fold_a)

            # self-loop term: ydinv = y * dinv; qd = (passA + ydinv) * dinv
            nc.vector.tensor_tensor(
                out=ydinv_sb[:].rearrange("p (t f) -> p t f", f=4),
                in0=y_sb[:].rearrange("p (t f) -> p t f", f=4),
                in1=dinv_sb[:].to_broadcast([P, NT, 4]),
                op=mybir.AluOpType.mult,
            )

            def emit_c_split(k):
                t0, t1 = SPLITS[k], SPLITS[k + 1]
                nc.sync.dma_start(
                    out=c_shard[t0 * P:t1 * P, :].rearrange(
                        "(t p) f -> p t f", p=P),
                    in_=c_sb[:, t0 * 4:t1 * 4].rearrange(
                        "p (t f) -> p t f", f=4),
                )
                if os.environ.get("K_NOCC"):
                    nc.sync.dma_start(out=c_fulls[k][0:P, :],
                                      in_=c_sb[:, t1 * 4 - 4:t1 * 4])
                else:
                    nc.gpsimd.collective_compute(
                        "AllGather", mybir.AluOpType.bypass,
                        replica_groups=[list(range(NC))],
                        ins=[c_shard[t0 * P:t1 * P, :].opt()],
                        outs=[c_fulls[k][:].opt()],
                    )
                nc.sync.dma_start(
                    out=c_pad[:, 0:8].rearrange(
                        "(c r) w -> c r w", c=NC)[:, SPLITS[k] * 64:SPLITS[k + 1] * 64, :],
                    in_=c_fulls[k][:].rearrange(
                        "(c r q) f -> c r (q f)", c=NC, q=2),
                )

            # ---------------- Layer 2 pass-B ----------------
            if phases >= 2:
                emit_l2_gathers((2, 3))
                nc.vector.tensor_tensor(
                    out=qd_sb[:].rearrange("p (t f) -> p t f", f=4),
                    in0=partA_sb[:].rearrange("p (t f) -> p t f", f=4),
                    in1=dinv_sb[:].to_broadcast([P, NT, 4]),
                    op=mybir.AluOpType.mult,
                )
                nc.vector.tensor_tensor(
                    out=qd_sb[:], in0=qd_sb[:], in1=ydinv_sb[:],
                    op=mybir.AluOpType.add,
                )
                for k in range(4):
                    emit_l2_tiles(SPLITS[k], SPLITS[k + 1], (2, 3), fold_b)
                    emit_c_split(k)


            # ---------------- Decode ----------------
            if phases >= 3:
                g0 = g2p.tile([P, DC2 * 4], BF16, tag="dg0")
                g1_ = g2p.tile([P, DC2 * 4], BF16, tag="dg1")
                # e0 parity runs: buckets (0,0)+(0,1) even, (1,0)+(1,1) odd
                e0_runs = [(dbase[0], dcb[0] + dcb[1], 0), (dbase[2], dcb[2] + dcb[3], 1)]
                e1_runs = [(dbase[b], dcb[b], b % 2) for b in range(4)]
                for gt, di, runs in ((g0, di0_sb, e0_runs), (g1_, di1_sb, e1_runs)):
                    gv = gt[:].rearrange("p (c e) -> p c e", e=4)
                    for base, cnt, par in runs:
                        if cnt == 0:
                            continue
                        _dma_gather_small(
                            nc.gpsimd,
                            out_ap=gv[:, base:base + cnt, :],
                            in_ap=c_pad[:, par * 4:par * 4 + 4],
                            idxs_ap=di[:, base * 8:(base + cnt) * 8],
                            num_idxs=cnt * P,
                            elem_size=4,
                            elem_step=128,
                        )
                nc.vector.tensor_tensor(
                    out=out_sb[:].rearrange("p (c e) -> p c e", e=2),
                    in0=g0[:].rearrange("p (c e) -> p c e", e=4)[:, :, 0:2],
                    in1=g1_[:].rearrange("p (c e) -> p c e", e=4)[:, :, 2:4],
                    op=mybir.AluOpType.add,
                )
            else:
                nc.vector.memset(out_sb[:], 0)

            nc.sync.dma_start(out=out_ap[:], in_=out_sb[:])

    nc.compile()
    return nc


def kernel(x, edge_index, pos_edge_index, neg_edge_index, W1, W2, Wlin):
    x = np.asarray(x, np.float32)
    W1 = np.asarray(W1, np.float32)
    W2 = np.asarray(W2, np.float32)
    Wlin = np.asarray(Wlin, np.float32)

    per_core, shape = _preprocess(x, edge_index, pos_edge_index, neg_edge_index)

    # fold W2 and Wlin: cols 0,1 pair with e0 (Wlin[:, :128]), cols 2,3 with e1
    Wl = np.stack([Wlin[0, :H], Wlin[1, :H], Wlin[0, H:], Wlin[1, H:]], axis=1)
    W2p = (W2 @ Wl).astype(np.float32)

    # xq = int8 row-quantized dinv * x (scale rides in the one-hot)
    ei = np.asarray(edge_index)
    dst = np.concatenate([ei[1], np.arange(N)]).astype(np.int64)
    deg = np.bincount(dst, minlength=N).astype(np.float64)
    dinv = 1.0 / np.sqrt(deg)
    xd = x * dinv[:, None].astype(np.float32)
    s_node = np.abs(xd).max(axis=1) / 127.0
    xq = np.zeros((N, 256), np.int8)
    xq[:, 0:H] = np.round(xd / s_node[:, None]).astype(np.int8)

    nc = _build_program(shape)

    s1, s2 = shape["l1"], shape["l2"]
    NCH1, NCH2, DC2 = s1["nch"], s2["nch"], shape["dc2"]
    in_maps = []
    for c in range(NC):
        m = per_core[c]
        iblob1 = m["idx1"]
        iblob2 = np.concatenate([m["idx2"], m["d_idx0"], m["d_idx1"]], axis=1)
        fblob = np.concatenate(
            [m["dstl1"], m["scl1"], m["dstl2"], m["dinv_t"], m["dinv2_t"]], axis=1
        ).astype(np.float32)
        in_maps.append(dict(xq=xq, w1=W1, w2p=W2p, iblob1=iblob1,
                            iblob2=iblob2, fblob=fblob))

    res = run_bass_kernel_spmd(nc, in_maps, core_ids=list(range(NC)))

    out = np.empty((NE_EVAL, 2), np.float32)
    for c in range(NC):
        shard = res.results[c]["out"]  # [P, DC2*2]; slot (p, k) at [p, 2k:2k+2]
        order, slot = per_core[c]["unperm"]
        vals = shard.reshape(P, DC2, 2)[slot % P, slot // P]
        out[c * EV_PC + order] = vals
    return out


# revision 55
# speedup vs baseline: 1.0445x; 1.0445x over previous
"""Trainium2 Bass kernel for a 2-layer GCN link predictor (NetLinkTrain).

Math: z = relu(A @ (x @ W1)); z2 = A @ (z @ W2); out = [z2[e0], z2[e1]] @ Wlin.T
where A = D^-1/2 (Adj + I) D^-1/2.

Key algebraic factorizations:
  * W2/Wlin folding: W2' = W2 @ [Wlin[:, :128].T | Wlin[:, 128:].T] ([128, 4]),
    so layer 2 and the decode work on 4-wide node features.
  * The symmetric norm dinv[src]*dinv[dst] factors entirely out of both
    aggregations: gather from xd = dinv*x, then
      z = dinv[dst] * relu(W1^T @ sum xd[src])        (relu commutes, dinv>0)
      y = dinv[dst]^2 * (z_hat @ W2')                 (folds both dst factors)
      c = dinv[dst] * sum y[src]
    so the scatter one-hots are pure (iota == dst) compares with no weights.

Data movement (the memory-bound core of the problem):
  * L1 gathers 256B bf16 xd rows per edge (dma_gather, 1 desc/edge).
  * L2/decode gather ONLY the 4 bf16 values needed per edge (8B elements at a
    256B-aligned element stride from a padded table, 2 nodes per 256B row).
    Sub-256B elements need a patched dma_gather builder (the stock bass
    assert requires elem_size%256B; the hardware only requires the element
    STRIDE to be a 256B multiple -- verified on device).
  * Aggregation per 128-dst tile: TensorE scatter matmul with a DVE-built
    one-hot, accumulating in PSUM.

Sharding: edges are sharded by destination-node range (core c owns nodes
[c*6250, (c+1)*6250)); each core owns its segment sums completely, so the only
collectives are two small AllGathers (y and c, ~50-100KB per core).

Host (numpy) does index preprocessing only: self-loops, degrees, sorting edges
by (dst tile, table class), wrapped int16 index layout, and the xd cast.
"""

import math
import os
import sys

import numpy as np

sys.path.insert(0, "/opt/trn_rl_repo")

import concourse.bacc as bacc
import concourse.bass as bass
import concourse.tile as tile
from concourse import ap_utils, mybir
from concourse.bass_utils import run_bass_kernel_spmd

N = 50000
H = 128
P = 128
NC = 8
NPC = N // NC              # 6250 nodes per core
NT = math.ceil(NPC / P)    # 49 dst tiles per core
PADN = NT * P              # 6272 padded nodes per core
NSLOT = NC * PADN          # 50176 packed slots
LO = 25000                 # x-table half split (int16 index limit)
G1 = 4                     # L1 tiles per gather call group
G2 = 13                    # L2 tiles per gather call group
NE_EVAL = 200000
EV_PC = NE_EVAL // NC      # 25000 eval edges per core

F32 = mybir.dt.float32
BF16 = mybir.dt.bfloat16
I16 = mybir.dt.int16
I8 = mybir.dt.int8

import ml_dtypes

BF16_NP = ml_dtypes.bfloat16


def _dma_gather_small(g, out_ap, in_ap, idxs_ap, num_idxs, elem_size, elem_step,
                      queue_num=0):
    """bass.BassGpSimd.dma_gather (DRAM source, non-transpose) without the
    elem_size%256B assert. The element STRIDE (elem_step bytes) must still be
    a 256B multiple; sub-256B elem_size verified on hardware."""
    g._assert_queue_num(queue_num)
    assert idxs_ap.dtype == mybir.dt.int16
    assert in_ap.dtype == out_ap.dtype
    assert ap_utils.ap_is_contiguous(in_ap.ap[1:])
    assert ap_utils.ap_is_contiguous(out_ap.ap[1:])
    assert ap_utils.ap_is_contiguous(idxs_ap.ap[1:])
    assert in_ap.ap[-1][1] == out_ap.ap[-1][1] == elem_size
    assert in_ap.ap[0][0] == elem_step
    stride_bytes = elem_step * mybir.dt.size(in_ap.dtype)
    stride_bytes_256, rem = divmod(stride_bytes, 256)
    assert rem == 0 and stride_bytes_256 < 256
    _in_ap = g.lower_ap_dma(in_ap, for_custom_bir_dma=True)
    return g.add_instruction(
        mybir.InstDMAGatherAnt(
            name=g.bass.get_next_instruction_name(),
            ins=[*_in_ap, g.lower_ap(idxs_ap),
                 g.lower_val_access(g.to_reg(num_idxs))],
            outs=[g.lower_ap(out_ap)],
            transpose=False,
            num_idxs=num_idxs,
            elem_size=elem_size,
            stride_bytes_256=stride_bytes_256,
            gen_mode=0,
            single_packet=False,
            queue_num=queue_num,
            sbuf_tokens_per_rank=0,
            sbuf_free_dim_per_rank=0,
            sbuf_free_dim_pad_per_rank=0,
            sbuf_byte_offset=0,
        )
    )


def _packed_id(n):
    """Packed slot of node n in the AllGathered y/c buffers: core-major,
    then natural node order (tile-major)."""
    n = np.asarray(n)
    c = n // NPC
    return c * PADN + (n - c * NPC)


def _wrap_idx(v, n_chunks):
    """v: [n_chunks, 128] int -> dma_gather wrapped idx layout [128, n_chunks*8]."""
    a16 = v.reshape(n_chunks, 8, 16).transpose(2, 0, 1).reshape(16, n_chunks * 8)
    return np.tile(a16, (8, 1)).astype(np.int16)


def _edge_layout(core, tl, cls, dloc, idxval, group_sz, sval=None, ncls=2):
    """Lay out edges into (tile, class)-grouped 128-slot chunks.

    Gather-call contiguity: chunks ordered group-major; within a group all
    class-k chunks (tile-major) for k = 0..ncls-1.

    Returns (idx_wrapped, dstl[, scale]) per core plus shared shape info.
    """
    key = (core * NT + tl) * ncls + cls
    counts = np.zeros((NC, NT, ncls), np.int64)
    np.add.at(counts, (core, tl, cls), 1)
    nch_tc = -(-counts.max(axis=0) // P)  # [NT, ncls] chunks per (tile, class)

    ngrp = math.ceil(NT / group_sz)
    chunk_start = np.zeros((NT, ncls), np.int64)
    grp_runs = [[] for _ in range(ngrp)]  # per group: [(base, count)] per class
    pos = 0
    for g in range(ngrp):
        t0, t1 = g * group_sz, min((g + 1) * group_sz, NT)
        for k in range(ncls):
            base = pos
            for t in range(t0, t1):
                chunk_start[t, k] = pos
                pos += nch_tc[t, k]
            grp_runs[g].append((base, pos - base))
    grp_lo = [r[0] for r in grp_runs]
    grp_hi = [r[1] for r in grp_runs]
    nch = pos

    order = np.argsort(key, kind="stable")
    s_key = key[order]
    group_start = np.concatenate([[0], np.cumsum(counts.reshape(-1))])[:-1]
    rank = np.arange(len(order)) - group_start[s_key]
    s_core = s_key // (NT * ncls)
    s_t = (s_key // ncls) % NT
    s_cls = s_key % ncls
    dest = chunk_start[s_t, s_cls] * P + rank

    per_core = []
    for c in range(NC):
        m = s_core == c
        slot_idx = np.zeros(nch * P, np.int64)
        slot_dstl = np.full(nch * P, 255.0, np.float32)
        d = dest[m]
        slot_idx[d] = idxval[order][m]
        slot_dstl[d] = dloc[order][m]
        ent = [_wrap_idx(slot_idx.reshape(nch, P), nch),
               slot_dstl.reshape(nch, P).T.copy()]
        if sval is not None:
            slot_s = np.zeros(nch * P, np.float32)
            slot_s[d] = sval[order][m]
            ent.append(slot_s.reshape(nch, P).T.copy())
        per_core.append(tuple(ent))
    shape = dict(nch=nch, nch_tc=nch_tc.tolist(), chunk_start=chunk_start.tolist(),
                 grp_lo=grp_lo, grp_hi=grp_hi, grp_runs=grp_runs)
    return per_core, shape


def _preprocess(x, edge_index, pos_edge_index, neg_edge_index):
    x = np.asarray(x, np.float32)
    ei = np.asarray(edge_index)
    src = np.concatenate([ei[0], np.arange(N)]).astype(np.int64)
    dst = np.concatenate([ei[1], np.arange(N)]).astype(np.int64)
    deg = np.bincount(dst, minlength=N).astype(np.float64)
    dinv = 1.0 / np.sqrt(deg)  # every node has a self loop -> deg >= 1

    core = dst // NPC
    dl = dst - core * NPC
    tl = dl // P
    dloc = (dl % P).astype(np.float32)

    # L1: class = src parity (even/odd 256B rows of the int8 x table);
    # per-slot int8 dequant scale rides in the one-hot's second op
    xd = x.astype(np.float32) * dinv[:, None].astype(np.float32)
    s_node = (np.abs(xd).max(axis=1) / 127.0).astype(np.float32)
    l1, shape1 = _edge_layout(core, tl, src % 2, dloc, src // 2, G1,
                              sval=s_node[src])
    # L2: 4 classes = (src tile-half, slot parity); half-A sources (local
    # tiles 0-24) live in a separate padded table that is complete mid-L1,
    # so half-A aggregation overlaps the end of layer 1. The appended
    # self-loops are excluded -- their contribution is added from the local
    # y shard on-device (exact, no gather).
    ne_real = ei.shape[1]
    score = src // NPC
    soff = src - score * NPC
    in_b = (soff >= 3200).astype(np.int64)
    row = np.where(in_b == 0, score * 1600 + soff // 2,
                   score * 1536 + (soff - 3200) // 2)
    cls4 = in_b * 2 + soff % 2
    l2, shape2 = _edge_layout(core[:ne_real], tl[:ne_real], cls4[:ne_real],
                              dloc[:ne_real], row[:ne_real], G2, ncls=4)

    # per-(p, t) dinv and dinv^2 for the post-aggregation scales
    offs = np.arange(NT * P).reshape(NT, P)  # off = t*128 + p
    dinv_t = np.zeros((NC, P, NT), np.float32)
    dinv2_t = np.zeros((NC, P, NT), np.float32)
    for c in range(NC):
        v = np.zeros(NT * P)
        vv = dinv[c * NPC:(c + 1) * NPC]
        v[:NPC] = vv
        dinv_t[c] = v[offs].T
        v[:NPC] = vv * vv
        dinv2_t[c] = v[offs].T

    # decode: bucket eval edges by (slot(e0)%2, slot(e1)%2)
    e0 = np.concatenate([np.asarray(pos_edge_index[0]), np.asarray(neg_edge_index[0])])
    e1 = np.concatenate([np.asarray(pos_edge_index[1]), np.asarray(neg_edge_index[1])])
    s0, s1 = _packed_id(e0), _packed_id(e1)
    bkt = (s0 % 2) * 2 + (s1 % 2)
    bcnt = np.zeros((NC, 4), np.int64)
    for c in range(NC):
        bcnt[c] = np.bincount(bkt[c * EV_PC:(c + 1) * EV_PC], minlength=4)
    dcb = (-(-bcnt.max(axis=0) // P)).tolist()  # chunks per bucket (shared)
    dbase = np.concatenate([[0], np.cumsum(dcb)]).tolist()
    dc2 = int(dbase[-1])

    dec = []
    for c in range(NC):
        sl = slice(c * EV_PC, (c + 1) * EV_PC)
        b = bkt[sl]
        order = np.argsort(b, kind="stable")
        rank = np.arange(EV_PC) - np.concatenate([[0], np.cumsum(bcnt[c])])[:-1][b[order]]
        slot = np.array(dbase)[b[order]] * P + rank
        i0 = np.zeros(dc2 * P, np.int64)
        i1 = np.zeros(dc2 * P, np.int64)
        i0[slot] = (s0[sl] // 2)[order]
        i1[slot] = (s1[sl] // 2)[order]
        dec.append(dict(
            d_idx0=_wrap_idx(i0.reshape(dc2, P), dc2),
            d_idx1=_wrap_idx(i1.reshape(dc2, P), dc2),
            # slot -> position in this core's eval range
            unperm=(np.asarray(order), slot),
        ))

    per_core = []
    for c in range(NC):
        per_core.append(dict(
            idx1=l1[c][0], dstl1=l1[c][1], scl1=l1[c][2],
            idx2=l2[c][0], dstl2=l2[c][1],
            dinv_t=dinv_t[c], dinv2_t=dinv2_t[c],
            d_idx0=dec[c]["d_idx0"], d_idx1=dec[c]["d_idx1"],
            unperm=dec[c]["unperm"],
        ))
    shape = dict(l1=shape1, l2=shape2, dcb=dcb, dbase=dbase, dc2=dc2)
    return per_core, shape


def _build_program(shape):
    s1, s2 = shape["l1"], shape["l2"]
    NCH1, NCH2, DC2 = s1["nch"], s2["nch"], shape["dc2"]
    dbase, dcb = shape["dbase"], shape["dcb"]
    mx1 = max(max(r) for r in s1["nch_tc"])
    mx2 = max(max(r) for r in s2["nch_tc"])
    shape_l1_grp = [
        (s1["grp_lo"][g][0], s1["grp_hi"][g][0] + s1["grp_hi"][g][1])
        for g in range(len(s1["grp_lo"]))
    ]

    nc = bacc.Bacc("TRN2", target_bir_lowering=False, debug=False, num_devices=NC)

    xq_ap = nc.dram_tensor("xq", [N, 256], I8, kind="ExternalInput").ap()
    w1_ap = nc.dram_tensor("w1", [H, H], F32, kind="ExternalInput").ap()
    w2p_ap = nc.dram_tensor("w2p", [H, 4], F32, kind="ExternalInput").ap()
    # metadata blobs: one int16, one f32
    IW1 = NCH1 * 8
    IW2 = (NCH2 + 2 * DC2) * 8
    FW = 2 * NCH1 + NCH2 + 2 * NT
    iblob1_ap = nc.dram_tensor("iblob1", [P, IW1], I16, kind="ExternalInput").ap()
    iblob2_ap = nc.dram_tensor("iblob2", [P, IW2], I16, kind="ExternalInput").ap()
    fblob_ap = nc.dram_tensor("fblob", [P, FW], F32, kind="ExternalInput").ap()
    out_ap = nc.dram_tensor("out", [P, DC2 * 2], F32, kind="ExternalOutput").ap()

    phases = int(os.environ.get("K_PHASES", "3"))

    with tile.TileContext(nc) as tc:
        with (
            tc.tile_pool(name="persist", bufs=1) as pp,
            tc.tile_pool(name="g1", bufs=6) as g1p,
            tc.tile_pool(name="g2", bufs=3) as g2p,
            tc.tile_pool(name="oh", bufs=12) as ohp,
            tc.tile_pool(name="oh2", bufs=24) as ohp2,
            tc.tile_pool(name="small", bufs=8) as sp,
            tc.tile_pool(name="psA", bufs=4, space="PSUM") as psA,
            tc.tile_pool(name="psB", bufs=2, space="PSUM") as psB,
            tc.tile_pool(name="psC", bufs=2, space="PSUM") as psC,
            tc.tile_pool(name="dram", bufs=1, space="DRAM") as dp,
        ):
            # ---- persistent metadata ----
            iblob1 = pp.tile([P, IW1], I16)
            iblob2 = pp.tile([P, IW2], I16)
            fblob = pp.tile([P, FW], F32)
            for _g in range(math.ceil(NT / G1)):
                _lo = shape_l1_grp[_g][0] * 8
                _hi = shape_l1_grp[_g][1] * 8
                nc.sync.dma_start(out=iblob1[:, _lo:_hi], in_=iblob1_ap[:, _lo:_hi])
                if _g == 0:
                    nc.sync.dma_start(out=fblob[:], in_=fblob_ap[:])
            idx1_sb = iblob1[:]
            idx2_sb = iblob2[:, 0:NCH2 * 8]
            di0_sb = iblob2[:, NCH2 * 8:(NCH2 + DC2) * 8]
            di1_sb = iblob2[:, (NCH2 + DC2) * 8:]
            dstl1_sb = fblob[:, 0:NCH1]
            scl1_sb = fblob[:, NCH1:2 * NCH1]
            dstl2_sb = fblob[:, 2 * NCH1:2 * NCH1 + NCH2]
            dinv_sb = fblob[:, 2 * NCH1 + NCH2:2 * NCH1 + NCH2 + NT]
            dinv2_sb = fblob[:, 2 * NCH1 + NCH2 + NT:]

            w1f = pp.tile([H, H], F32)
            w2pf = pp.tile([H, 4], F32)
            nc.sync.dma_start(out=w1f[:], in_=w1_ap[:])
            nc.sync.dma_start(out=w2pf[:], in_=w2p_ap[:])
            w1b = pp.tile([H, H], BF16)
            w2pb = pp.tile([H, 4], BF16)
            nc.vector.tensor_copy(out=w1b[:], in_=w1f[:])
            nc.vector.tensor_copy(out=w2pb[:], in_=w2pf[:])

            iota_f = pp.tile([P, P], BF16)
            nc.gpsimd.iota(iota_f[:], pattern=[[1, P]], base=0,
                           channel_multiplier=0,
                           allow_small_or_imprecise_dtypes=True)

            y_sb = pp.tile([P, NT * 4], BF16)
            ydinv_sb = pp.tile([P, NT * 4], BF16)
            partA_sb = pp.tile([P, NT * 4], F32)
            qd_sb = pp.tile([P, NT * 4], F32)
            c_sb = pp.tile([P, NT * 4], BF16)
            out_sb = pp.tile([P, DC2 * 2], F32)

            SPLITS = [0, 13, 25, 37, NT]
            y_shard = dp.tile([PADN, 4], BF16)
            y_padA = dp.tile([NC * 1600, 128], BF16)
            y_padB = dp.tile([NC * 1536, 128], BF16)
            c_shard = dp.tile([PADN, 4], BF16)
            c_pad = dp.tile([NSLOT // 2, 128], BF16)
            y_fulls, c_fulls = [], []
            for k in range(4):
                nk = NC * (SPLITS[k + 1] - SPLITS[k]) * P
                y_fulls.append(dp.tile([nk, 4], BF16, name=f"y_full{k}"))
                c_fulls.append(dp.tile([nk, 4], BF16, name=f"c_full{k}"))

            xq2 = xq_ap[:].rearrange("(a b) e -> a b e", b=2)
            x_lo = xq2[:, 0, 0:H]
            x_hi = xq2[:, 1, 0:H]

            def build_oh(dstl_sb, c0, cnt, scl_sb=None, pool=None, mx=16,
                         act_nd=None):
                oh = (pool or ohp).tile([P, mx * P], BF16, tag="oh")
                for j in range(cnt):
                    if act_nd is not None and j % 4 == 0:
                        # ACT two-pass: a = |iota - d|; oh = relu(1 - a)
                        a = sp.tile([P, P], BF16, tag="oha")
                        nc.scalar.activation(
                            out=a[:], in_=iota_f[:],
                            func=mybir.ActivationFunctionType.Abs,
                            bias=act_nd[:, c0 + j:c0 + j + 1])
                        nc.scalar.activation(
                            out=oh[:, j * P:(j + 1) * P], in_=a[:],
                            func=mybir.ActivationFunctionType.Relu,
                            bias=1.0, scale=-1.0)
                        continue
                    nc.vector.tensor_scalar(
                        out=oh[:, j * P:(j + 1) * P],
                        in0=iota_f[:],
                        scalar1=dstl_sb[:, c0 + j:c0 + j + 1],
                        scalar2=(scl_sb[:, c0 + j:c0 + j + 1]
                                 if scl_sb is not None else None),
                        op0=mybir.AluOpType.is_equal,
                        op1=(mybir.AluOpType.mult
                             if scl_sb is not None else mybir.AluOpType.bypass),
                    )
                return oh[:, :cnt * P].rearrange("p (c e) -> p c e", e=P)

            # ---------------- Layer 1 + overlapped L2 pass-A ----------------
            Y_OFF_A = [0, 13 * 64]    # repad row offsets in y_padA (tiles 0-12, 13-24)
            Y_OFF_B = [0, 12 * 64]    # in y_padB (tiles 25-36, 37-48)

            def emit_y_split(k):
                t0, t1 = SPLITS[k], SPLITS[k + 1]
                nt = t1 - t0
                nc.sync.dma_start(
                    out=y_shard[t0 * P:t1 * P, :].rearrange(
                        "(t p) f -> p t f", p=P),
                    in_=y_sb[:, t0 * 4:t1 * 4].rearrange("p (t f) -> p t f", f=4),
                )
                if os.environ.get("K_NOCC"):
                    nc.sync.dma_start(out=y_fulls[k][0:P, :],
                                      in_=y_sb[:, t1 * 4 - 4:t1 * 4])
                else:
                    nc.gpsimd.collective_compute(
                        "AllGather", mybir.AluOpType.bypass,
                        replica_groups=[list(range(NC))],
                        ins=[y_shard[t0 * P:t1 * P, :].opt()],
                        outs=[y_fulls[k][:].opt()],
                    )
                pad, off = (y_padA, Y_OFF_A[k]) if k < 2 else (y_padB, Y_OFF_B[k - 2])
                nc.sync.dma_start(
                    out=pad[:, 0:8].rearrange(
                        "(c r) w -> c r w", c=NC)[:, off:off + nt * 64, :],
                    in_=y_fulls[k][:].rearrange(
                        "(c r q) f -> c r (q f)", c=NC, q=2),
                )

            gath2 = pp.tile([P, NCH2 * 4], BF16)
            g3d2 = gath2[:].rearrange("p (c e) -> p c e", e=4)

            def emit_l2_gathers(classes):
                for g in range(math.ceil(NT / G2)):
                    for k in classes:
                        base, cnt = s2["grp_runs"][g][k]
                        if cnt == 0:
                            continue
                        pad = y_padA if k < 2 else y_padB
                        _dma_gather_small(
                            nc.gpsimd,
                            out_ap=g3d2[:, base:base + cnt, :],
                            in_ap=pad[:, (k % 2) * 4:(k % 2) * 4 + 4],
                            idxs_ap=idx2_sb[:, base * 8:(base + cnt) * 8],
                            num_idxs=cnt * P,
                            elem_size=4,
                            elem_step=128,
                        )

            def emit_l2_tiles(t0, t1, classes, fold):
                for t in range(t0, t1):
                    runs = [(s2["chunk_start"][t][k], s2["nch_tc"][t][k])
                            for k in classes]
                    cnt = sum(cn for _, cn in runs)
                    if classes == (0, 1) and t in ohA_pre:
                        ohs = ohA_pre.pop(t)
                    else:
                        ohs = [build_oh(dstl2_sb, cb, cn, pool=ohp2, mx=mx2)
                               if cn else None for cb, cn in runs]
                    c_ps = psC.tile([P, 4], F32, tag="p4")
                    kk = 0
                    for (cb, cn), o3 in zip(runs, ohs):
                        for j in range(cn):
                            nc.tensor.matmul(
                                out=c_ps[:], lhsT=o3[:, j, :],
                                rhs=g3d2[:, cb + j, :],
                                start=(kk == 0), stop=(kk == cnt - 1),
                            )
                            kk += 1
                    fold(t, c_ps)

            def fold_a(t, c_ps):
                nc.scalar.copy(out=partA_sb[:, t * 4:t * 4 + 4], in_=c_ps[:])

            def fold_b(t, c_ps):
                nc.vector.scalar_tensor_tensor(
                    out=c_sb[:, t * 4:t * 4 + 4], in0=c_ps[:],
                    scalar=dinv_sb[:, t:t + 1],
                    in1=qd_sb[:, t * 4:t * 4 + 4],
                    op0=mybir.AluOpType.mult,
                    op1=mybir.AluOpType.add,
                )

            ohA_pre = {}

            def prebuild_passA(t0, t1):
                for t in range(t0, t1):
                    runs = [(s2["chunk_start"][t][k], s2["nch_tc"][t][k])
                            for k in (0, 1)]
                    ohA_pre[t] = [build_oh(dstl2_sb, cb, cn, pool=ohp2, mx=mx2)
                                  if cn else None for cb, cn in runs]

            ngrp1 = math.ceil(NT / G1)
            g1max = max(s1["grp_lo"][g][1] + s1["grp_hi"][g][1] for g in range(ngrp1))
            for g in range(ngrp1):
                if g == ngrp1 - 1:
                    nc.sync.dma_start(out=iblob2[:], in_=iblob2_ap[:])
                if phases >= 2 and g * G1 >= 28 and (g - 1) * G1 < 28:
                    prebuild_passA(0, 3)
                if phases >= 2 and g * G1 >= 40 and (g - 1) * G1 < 40:
                    prebuild_passA(3, 6)
                lo_b, lo_n = s1["grp_lo"][g]
                hi_b, hi_n = s1["grp_hi"][g]
                gcnt = lo_n + hi_n
                gq = g1p.tile([P, g1max * H], I8, tag="g1q")
                q3d = gq[:, :gcnt * H].rearrange("p (c e) -> p c e", e=H)
                for base, cnt, table in ((lo_b, lo_n, x_lo), (hi_b, hi_n, x_hi)):
                    if cnt == 0:
                        continue
                    _dma_gather_small(
                        nc.gpsimd,
                        out_ap=q3d[:, base - lo_b:base - lo_b + cnt, :],
                        in_ap=table,
                        idxs_ap=idx1_sb[:, base * 8:(base + cnt) * 8],
                        num_idxs=cnt * P,
                        elem_size=H,
                        elem_step=512,
                    )

                for t in range(g * G1, min((g + 1) * G1, NT)):
                    cl0, cn0 = s1["chunk_start"][t][0], s1["nch_tc"][t][0]
                    ch0, cn1 = s1["chunk_start"][t][1], s1["nch_tc"][t][1]
                    cnt = cn0 + cn1
                    msgs = sp.tile([P, 16 * H], BF16, tag="msgs")
                    nc.scalar.copy(out=msgs[:, :cn0 * H],
                                   in_=q3d[:, cl0 - lo_b:cl0 - lo_b + cn0, :])
                    if cn1:
                        nc.scalar.copy(out=msgs[:, cn0 * H:cnt * H],
                                       in_=q3d[:, ch0 - lo_b:ch0 - lo_b + cn1, :])
                    m3d = msgs[:, :cnt * H].rearrange("p (c e) -> p c e", e=H)
                    oh = build_oh(dstl1_sb, cl0, cn0, scl1_sb, mx=mx1)
                    oh2 = (build_oh(dstl1_sb, ch0, cn1, scl1_sb, mx=mx1)
                           if cn1 else None)
                    ut_ps = psA.tile([P, P], F32, tag="ut")
                    k = 0
                    for (cb, cn, o3) in ((cl0, cn0, oh), (ch0, cn1, oh2)):
                        for j in range(cn):
                            nc.tensor.matmul(
                                out=ut_ps[:], lhsT=m3d[:, k, :], rhs=o3[:, j, :],
                                start=(k == 0), stop=(k == cnt - 1),
                            )
                            k += 1
                    ut_sb = sp.tile([P, P], BF16, tag="utsb")
                    nc.scalar.copy(out=ut_sb[:], in_=ut_ps[:])
                    vt_ps = psB.tile([P, P], F32, tag="vt")
                    nc.tensor.matmul(out=vt_ps[:], lhsT=w1b[:], rhs=ut_sb[:],
                                     start=True, stop=True)
                    zt_sb = sp.tile([P, P], BF16, tag="ztsb")
                    nc.vector.tensor_relu(out=zt_sb[:], in_=vt_ps[:])
                    y_ps = psC.tile([P, 4], F32, tag="p4")
                    nc.tensor.matmul(out=y_ps[:], lhsT=zt_sb[:], rhs=w2pb[:],
                                     start=True, stop=True)
                    nc.vector.tensor_scalar(
                        out=y_sb[:, t * 4:t * 4 + 4], in0=y_ps[:],
                        scalar1=dinv2_sb[:, t:t + 1], scalar2=None,
                        op0=mybir.AluOpType.mult,
                    )

            for k in range(4):
                emit_y_split(k)
            if phases >= 2:
                emit_l2_gathers((0, 1))
                emit_l2_tiles(0, NT, (0, 1), fold_a)

            # self-loop term: ydinv = y * dinv; qd = (passA + ydinv) * dinv
            nc.vector.tensor_tensor(
                out=ydinv_sb[:].rearrange("p (t f) -> p t f", f=4),
                in0=y_sb[:].rearrange("p (t f) -> p t f", f=4),
                in1=dinv_sb[:].to_broadcast([P, NT, 4]),
                op=mybir.AluOpType.mult,
            )

            def emit_c_split(k):
                t0, t1 = SPLITS[k], SPLITS[k + 1]
                nc.sync.dma_start(
                    out=c_shard[t0 * P:t1 * P, :].rearrange(
                        "(t p) f -> p t f", p=P),
                    in_=c_sb[:, t0 * 4:t1 * 4].rearrange(
                        "p (t f) -> p t f", f=4),
                )
                if os.environ.get("K_NOCC"):
                    nc.sync.dma_start(out=c_fulls[k][0:P, :],
                                      in_=c_sb[:, t1 * 4 - 4:t1 * 4])
                else:
                    nc.gpsimd.collective_compute(
                        "AllGather", mybir.AluOpType.bypass,
                        replica_groups=[list(range(NC))],
                        ins=[c_shard[t0 * P:t1 * P, :].opt()],
                        outs=[c_fulls[k][:].opt()],
                    )
                nc.sync.dma_start(
                    out=c_pad[:, 0:8].rearrange(
                        "(c r) w -> c r w", c=NC)[:, SPLITS[k] * 64:SPLITS[k + 1] * 64, :],
                    in_=c_fulls[k][:].rearrange(
                        "(c r q) f -> c r (q f)", c=NC, q=2),
                )

            # ---------------- Layer 2 pass-B ----------------
            if phases >= 2:
                emit_l2_gathers((2, 3))
                nc.vector.tensor_tensor(
                    out=qd_sb[:].rearrange("p (t f) -> p t f", f=4),
                    in0=partA_sb[:].rearrange("p (t f) -> p t f", f=4),
                    in1=dinv_sb[:].to_broadcast([P, NT, 4]),
                    op=mybir.AluOpType.mult,
                )
                nc.vector.tensor_tensor(
                    out=qd_sb[:], in0=qd_sb[:], in1=ydinv_sb[:],
                    op=mybir.AluOpType.add,
                )
                for k in range(4):
                    emit_l2_tiles(SPLITS[k], SPLITS[k + 1], (2, 3), fold_b)
                    emit_c_split(k)


            # ---------------- Decode ----------------
            if phases >= 3:
                g0 = g2p.tile([P, DC2 * 4], BF16, tag="dg0")
                g1_ = g2p.tile([P, DC2 * 4], BF16, tag="dg1")
                # e0 parity runs: buckets (0,0)+(0,1) even, (1,0)+(1,1) odd
                e0_runs = [(dbase[0], dcb[0] + dcb[1], 0), (dbase[2], dcb[2] + dcb[3], 1)]
                e1_runs = [(dbase[b], dcb[b], b % 2) for b in range(4)]
                for gt, di, runs in ((g0, di0_sb, e0_runs), (g1_, di1_sb, e1_runs)):
                    gv = gt[:].rearrange("p (c e) -> p c e", e=4)
                    for base, cnt, par in runs:
                        if cnt == 0:
                            continue
                        _dma_gather_small(
                            nc.gpsimd,
                            out_ap=gv[:, base:base + cnt, :],
                            in_ap=c_pad[:, par * 4:par * 4 + 4],
                            idxs_ap=di[:, base * 8:(base + cnt) * 8],
                            num_idxs=cnt * P,
                            elem_size=4,
                            elem_step=128,
                        )
                nc.vector.tensor_tensor(
                    out=out_sb[:].rearrange("p (c e) -> p c e", e=2),
                    in0=g0[:].rearrange("p (c e) -> p c e", e=4)[:, :, 0:2],
                    in1=g1_[:].rearrange("p (c e) -> p c e", e=4)[:, :, 2:4],
                    op=mybir.AluOpType.add,
                )
            else:
                nc.vector.memset(out_sb[:], 0)

            nc.sync.dma_start(out=out_ap[:], in_=out_sb[:])

    nc.compile()
    return nc


def kernel(x, edge_index, pos_edge_index, neg_edge_index, W1, W2, Wlin):
    x = np.asarray(x, np.float32)
    W1 = np.asarray(W1, np.float32)
    W2 = np.asarray(W2, np.float32)
    Wlin = np.asarray(Wlin, np.float32)

    per_core, shape = _preprocess(x, edge_index, pos_edge_index, neg_edge_index)

    # fold W2 and Wlin: cols 0,1 pair with e0 (Wlin[:, :128]), cols 2,3 with e1
    Wl = np.stack([Wlin[0, :H], Wlin[1, :H], Wlin[0, H:], Wlin[1, H:]], axis=1)
    W2p = (W2 @ Wl).astype(np.float32)

    # xq = int8 row-quantized dinv * x (scale rides in the one-hot)
    ei = np.asarray(edge_index)
    dst = np.concatenate([ei[1], np.arange(N)]).astype(np.int64)
    deg = np.bincount(dst, minlength=N).astype(np.float64)
    dinv = 1.0 / np.sqrt(deg)
    xd = x * dinv[:, None].astype(np.float32)
    s_node = np.abs(xd).max(axis=1) / 127.0
    xq = np.zeros((N, 256), np.int8)
    xq[:, 0:H] = np.round(xd / s_node[:, None]).astype(np.int8)

    nc = _build_program(shape)

    s1, s2 = shape["l1"], shape["l2"]
    NCH1, NCH2, DC2 = s1["nch"], s2["nch"], shape["dc2"]
    in_maps = []
    for c in range(NC):
        m = per_core[c]
        iblob1 = m["idx1"]
        iblob2 = np.concatenate([m["idx2"], m["d_idx0"], m["d_idx1"]], axis=1)
        fblob = np.concatenate(
            [m["dstl1"], m["scl1"], m["dstl2"], m["dinv_t"], m["dinv2_t"]], axis=1
        ).astype(np.float32)
        in_maps.append(dict(xq=xq, w1=W1, w2p=W2p, iblob1=iblob1,
                            iblob2=iblob2, fblob=fblob))

    res = run_bass_kernel_spmd(nc, in_maps, core_ids=list(range(NC)))

    out = np.empty((NE_EVAL, 2), np.float32)
    for c in range(NC):
        shard = res.results[c]["out"]  # [P, DC2*2]; slot (p, k) at [p, 2k:2k+2]
        order, slot = per_core[c]["unperm"]
        vals = shard.reshape(P, DC2, 2)[slot % P, slot // P]
        out[c * EV_PC + order] = vals
    return out


# revision 56
# speedup vs baseline: 1.0547x; 1.0098x over previous
"""Trainium2 Bass kernel for a 2-layer GCN link predictor (NetLinkTrain).

Math: z = relu(A @ (x @ W1)); z2 = A @ (z @ W2); out = [z2[e0], z2[e1]] @ Wlin.T
where A = D^-1/2 (Adj + I) D^-1/2.

Key algebraic factorizations:
  * W2/Wlin folding: W2' = W2 @ [Wlin[:, :128].T | Wlin[:, 128:].T] ([128, 4]),
    so layer 2 and the decode work on 4-wide node features.
  * The symmetric norm dinv[src]*dinv[dst] factors entirely out of both
    aggregations: gather from xd = dinv*x, then
      z = dinv[dst] * relu(W1^T @ sum xd[src])        (relu commutes, dinv>0)
      y = dinv[dst]^2 * (z_hat @ W2')                 (folds both dst factors)
      c = dinv[dst] * sum y[src]
    so the scatter one-hots are pure (iota == dst) compares with no weights.

Data movement (the memory-bound core of the problem):
  * L1 gathers 256B bf16 xd rows per edge (dma_gather, 1 desc/edge).
  * L2/decode gather ONLY the 4 bf16 values needed per edge (8B elements at a
    256B-aligned element stride from a padded table, 2 nodes per 256B row).
    Sub-256B elements need a patched dma_gather builder (the stock bass
    assert requires elem_size%256B; the hardware only requires the element
    STRIDE to be a 256B multiple -- verified on device).
  * Aggregation per 128-dst tile: TensorE scatter matmul with a DVE-built
    one-hot, accumulating in PSUM.

Sharding: edges are sharded by destination-node range (core c owns nodes
[c*6250, (c+1)*6250)); each core owns its segment sums completely, so the only
collectives are two small AllGathers (y and c, ~50-100KB per core).

Host (numpy) does index preprocessing only: self-loops, degrees, sorting edges
by (dst tile, table class), wrapped int16 index layout, and the xd cast.
"""

import math
import os
import sys

import numpy as np

sys.path.insert(0, "/opt/trn_rl_repo")

import concourse.bacc as bacc
import concourse.bass as bass
import concourse.tile as tile
from concourse import ap_utils, mybir
from concourse.bass_utils import run_bass_kernel_spmd

N = 50000
H = 128
P = 128
NC = 8
NPC = N // NC              # 6250 nodes per core
NT = math.ceil(NPC / P)    # 49 dst tiles per core
PADN = NT * P              # 6272 padded nodes per core
NSLOT = NC * PADN          # 50176 packed slots
LO = 25000                 # x-table half split (int16 index limit)
G1 = 4                     # L1 tiles per gather call group
G2 = 13                    # L2 tiles per gather call group
NE_EVAL = 200000
EV_PC = NE_EVAL // NC      # 25000 eval edges per core

F32 = mybir.dt.float32
BF16 = mybir.dt.bfloat16
I16 = mybir.dt.int16
I8 = mybir.dt.int8

import ml_dtypes

BF16_NP = ml_dtypes.bfloat16


def _dma_gather_small(g, out_ap, in_ap, idxs_ap, num_idxs, elem_size, elem_step,
                      queue_num=0):
    """bass.BassGpSimd.dma_gather (DRAM source, non-transpose) without the
    elem_size%256B assert. The element STRIDE (elem_step bytes) must still be
    a 256B multiple; sub-256B elem_size verified on hardware."""
    g._assert_queue_num(queue_num)
    assert idxs_ap.dtype == mybir.dt.int16
    assert in_ap.dtype == out_ap.dtype
    assert ap_utils.ap_is_contiguous(in_ap.ap[1:])
    assert ap_utils.ap_is_contiguous(out_ap.ap[1:])
    assert ap_utils.ap_is_contiguous(idxs_ap.ap[1:])
    assert in_ap.ap[-1][1] == out_ap.ap[-1][1] == elem_size
    assert in_ap.ap[0][0] == elem_step
    stride_bytes = elem_step * mybir.dt.size(in_ap.dtype)
    stride_bytes_256, rem = divmod(stride_bytes, 256)
    assert rem == 0 and stride_bytes_256 < 256
    _in_ap = g.lower_ap_dma(in_ap, for_custom_bir_dma=True)
    return g.add_instruction(
        mybir.InstDMAGatherAnt(
            name=g.bass.get_next_instruction_name(),
            ins=[*_in_ap, g.lower_ap(idxs_ap),
                 g.lower_val_access(g.to_reg(num_idxs))],
            outs=[g.lower_ap(out_ap)],
            transpose=False,
            num_idxs=num_idxs,
            elem_size=elem_size,
            stride_bytes_256=stride_bytes_256,
            gen_mode=0,
            single_packet=False,
            queue_num=queue_num,
            sbuf_tokens_per_rank=0,
            sbuf_free_dim_per_rank=0,
            sbuf_free_dim_pad_per_rank=0,
            sbuf_byte_offset=0,
        )
    )


def _packed_id(n):
    """Packed slot of node n in the AllGathered y/c buffers: core-major,
    then natural node order (tile-major)."""
    n = np.asarray(n)
    c = n // NPC
    return c * PADN + (n - c * NPC)


def _wrap_idx(v, n_chunks):
    """v: [n_chunks, 128] int -> dma_gather wrapped idx layout [128, n_chunks*8]."""
    a16 = v.reshape(n_chunks, 8, 16).transpose(2, 0, 1).reshape(16, n_chunks * 8)
    return np.tile(a16, (8, 1)).astype(np.int16)


def _edge_layout(core, tl, cls, dloc, idxval, group_sz, sval=None, ncls=2):
    """Lay out edges into (tile, class)-grouped 128-slot chunks.

    Gather-call contiguity: chunks ordered group-major; within a group all
    class-k chunks (tile-major) for k = 0..ncls-1.

    Returns (idx_wrapped, dstl[, scale]) per core plus shared shape info.
    """
    key = (core * NT + tl) * ncls + cls
    counts = np.zeros((NC, NT, ncls), np.int64)
    np.add.at(counts, (core, tl, cls), 1)
    nch_tc = -(-counts.max(axis=0) // P)  # [NT, ncls] chunks per (tile, class)

    ngrp = math.ceil(NT / group_sz)
    chunk_start = np.zeros((NT, ncls), np.int64)
    grp_runs = [[] for _ in range(ngrp)]  # per group: [(base, count)] per class
    pos = 0
    for g in range(ngrp):
        t0, t1 = g * group_sz, min((g + 1) * group_sz, NT)
        for k in range(ncls):
            base = pos
            for t in range(t0, t1):
                chunk_start[t, k] = pos
                pos += nch_tc[t, k]
            grp_runs[g].append((base, pos - base))
    grp_lo = [r[0] for r in grp_runs]
    grp_hi = [r[1] for r in grp_runs]
    nch = pos

    order = np.argsort(key, kind="stable")
    s_key = key[order]
    group_start = np.concatenate([[0], np.cumsum(counts.reshape(-1))])[:-1]
    rank = np.arange(len(order)) - group_start[s_key]
    s_core = s_key // (NT * ncls)
    s_t = (s_key // ncls) % NT
    s_cls = s_key % ncls
    dest = chunk_start[s_t, s_cls] * P + rank

    per_core = []
    for c in range(NC):
        m = s_core == c
        slot_idx = np.zeros(nch * P, np.int64)
        slot_dstl = np.full(nch * P, 255.0, np.float32)
        d = dest[m]
        slot_idx[d] = idxval[order][m]
        slot_dstl[d] = dloc[order][m]
        ent = [_wrap_idx(slot_idx.reshape(nch, P), nch),
               slot_dstl.reshape(nch, P).T.copy()]
        if sval is not None:
            slot_s = np.zeros(nch * P, np.float32)
            slot_s[d] = sval[order][m]
            ent.append(slot_s.reshape(nch, P).T.copy())
        per_core.append(tuple(ent))
    shape = dict(nch=nch, nch_tc=nch_tc.tolist(), chunk_start=chunk_start.tolist(),
                 grp_lo=grp_lo, grp_hi=grp_hi, grp_runs=grp_runs)
    return per_core, shape


def _preprocess(x, edge_index, pos_edge_index, neg_edge_index):
    x = np.asarray(x, np.float32)
    ei = np.asarray(edge_index)
    src = np.concatenate([ei[0], np.arange(N)]).astype(np.int64)
    dst = np.concatenate([ei[1], np.arange(N)]).astype(np.int64)
    deg = np.bincount(dst, minlength=N).astype(np.float64)
    dinv = 1.0 / np.sqrt(deg)  # every node has a self loop -> deg >= 1

    core = dst // NPC
    dl = dst - core * NPC
    tl = dl // P
    dloc = (dl % P).astype(np.float32)

    # L1: class = src parity (even/odd 256B rows of the int8 x table);
    # per-slot int8 dequant scale rides in the one-hot's second op
    xd = x.astype(np.float32) * dinv[:, None].astype(np.float32)
    s_node = (np.abs(xd).max(axis=1) / 127.0).astype(np.float32)
    l1, shape1 = _edge_layout(core, tl, src % 2, dloc, src // 2, G1,
                              sval=s_node[src])
    # L2: 4 classes = (src tile-half, slot parity); half-A sources (local
    # tiles 0-24) live in a separate padded table that is complete mid-L1,
    # so half-A aggregation overlaps the end of layer 1. The appended
    # self-loops are excluded -- their contribution is added from the local
    # y shard on-device (exact, no gather).
    ne_real = ei.shape[1]
    score = src // NPC
    soff = src - score * NPC
    in_b = (soff >= 3200).astype(np.int64)
    row = np.where(in_b == 0, score * 1600 + soff // 2,
                   score * 1536 + (soff - 3200) // 2)
    cls4 = in_b * 2 + soff % 2
    l2, shape2 = _edge_layout(core[:ne_real], tl[:ne_real], cls4[:ne_real],
                              dloc[:ne_real], row[:ne_real], G2, ncls=4)

    # per-(p, t) dinv and dinv^2 for the post-aggregation scales
    offs = np.arange(NT * P).reshape(NT, P)  # off = t*128 + p
    dinv_t = np.zeros((NC, P, NT), np.float32)
    dinv2_t = np.zeros((NC, P, NT), np.float32)
    for c in range(NC):
        v = np.zeros(NT * P)
        vv = dinv[c * NPC:(c + 1) * NPC]
        v[:NPC] = vv
        dinv_t[c] = v[offs].T
        v[:NPC] = vv * vv
        dinv2_t[c] = v[offs].T

    # decode: bucket eval edges by (slot(e0)%2, slot(e1)%2)
    e0 = np.concatenate([np.asarray(pos_edge_index[0]), np.asarray(neg_edge_index[0])])
    e1 = np.concatenate([np.asarray(pos_edge_index[1]), np.asarray(neg_edge_index[1])])
    s0, s1 = _packed_id(e0), _packed_id(e1)
    bkt = (s0 % 2) * 2 + (s1 % 2)
    bcnt = np.zeros((NC, 4), np.int64)
    for c in range(NC):
        bcnt[c] = np.bincount(bkt[c * EV_PC:(c + 1) * EV_PC], minlength=4)
    dcb = (-(-bcnt.max(axis=0) // P)).tolist()  # chunks per bucket (shared)
    dbase = np.concatenate([[0], np.cumsum(dcb)]).tolist()
    dc2 = int(dbase[-1])

    dec = []
    for c in range(NC):
        sl = slice(c * EV_PC, (c + 1) * EV_PC)
        b = bkt[sl]
        order = np.argsort(b, kind="stable")
        rank = np.arange(EV_PC) - np.concatenate([[0], np.cumsum(bcnt[c])])[:-1][b[order]]
        slot = np.array(dbase)[b[order]] * P + rank
        i0 = np.zeros(dc2 * P, np.int64)
        i1 = np.zeros(dc2 * P, np.int64)
        i0[slot] = (s0[sl] // 2)[order]
        i1[slot] = (s1[sl] // 2)[order]
        dec.append(dict(
            d_idx0=_wrap_idx(i0.reshape(dc2, P), dc2),
            d_idx1=_wrap_idx(i1.reshape(dc2, P), dc2),
            # slot -> position in this core's eval range
            unperm=(np.asarray(order), slot),
        ))

    per_core = []
    for c in range(NC):
        per_core.append(dict(
            idx1=l1[c][0], dstl1=l1[c][1], scl1=l1[c][2],
            idx2=l2[c][0], dstl2=l2[c][1],
            dinv_t=dinv_t[c], dinv2_t=dinv2_t[c],
            d_idx0=dec[c]["d_idx0"], d_idx1=dec[c]["d_idx1"],
            unperm=dec[c]["unperm"],
        ))
    shape = dict(l1=shape1, l2=shape2, dcb=dcb, dbase=dbase, dc2=dc2)
    return per_core, shape


def _build_program(shape):
    s1, s2 = shape["l1"], shape["l2"]
    NCH1, NCH2, DC2 = s1["nch"], s2["nch"], shape["dc2"]
    dbase, dcb = shape["dbase"], shape["dcb"]
    mx1 = max(max(r) for r in s1["nch_tc"])
    mx2 = max(max(r) for r in s2["nch_tc"])
    shape_l1_grp = [
        (s1["grp_lo"][g][0], s1["grp_hi"][g][0] + s1["grp_hi"][g][1])
        for g in range(len(s1["grp_lo"]))
    ]

    nc = bacc.Bacc("TRN2", target_bir_lowering=False, debug=False, num_devices=NC)

    xq_ap = nc.dram_tensor("xq", [N, 256], I8, kind="ExternalInput").ap()
    w1_ap = nc.dram_tensor("w1", [H, H], F32, kind="ExternalInput").ap()
    w2p_ap = nc.dram_tensor("w2p", [H, 4], F32, kind="ExternalInput").ap()
    # metadata blobs: one int16, one f32
    IW1 = NCH1 * 8
    IW2 = (NCH2 + 2 * DC2) * 8
    FW = 2 * NCH1 + NCH2 + 2 * NT
    iblob1_ap = nc.dram_tensor("iblob1", [P, IW1], I16, kind="ExternalInput").ap()
    iblob2_ap = nc.dram_tensor("iblob2", [P, IW2], I16, kind="ExternalInput").ap()
    fblob_ap = nc.dram_tensor("fblob", [P, FW], F32, kind="ExternalInput").ap()
    out_ap = nc.dram_tensor("out", [P, DC2 * 2], F32, kind="ExternalOutput").ap()

    phases = int(os.environ.get("K_PHASES", "3"))

    with tile.TileContext(nc) as tc:
        with (
            tc.tile_pool(name="persist", bufs=1) as pp,
            tc.tile_pool(name="g1", bufs=6) as g1p,
            tc.tile_pool(name="g2", bufs=3) as g2p,
            tc.tile_pool(name="oh", bufs=12) as ohp,
            tc.tile_pool(name="oh2", bufs=32) as ohp2,
            tc.tile_pool(name="small", bufs=8) as sp,
            tc.tile_pool(name="psA", bufs=4, space="PSUM") as psA,
            tc.tile_pool(name="psB", bufs=2, space="PSUM") as psB,
            tc.tile_pool(name="psC", bufs=2, space="PSUM") as psC,
            tc.tile_pool(name="dram", bufs=1, space="DRAM") as dp,
        ):
            # ---- persistent metadata ----
            iblob1 = pp.tile([P, IW1], I16)
            iblob2 = pp.tile([P, IW2], I16)
            fblob = pp.tile([P, FW], F32)
            for _g in range(math.ceil(NT / G1)):
                _lo = shape_l1_grp[_g][0] * 8
                _hi = shape_l1_grp[_g][1] * 8
                nc.sync.dma_start(out=iblob1[:, _lo:_hi], in_=iblob1_ap[:, _lo:_hi])
                if _g == 0:
                    nc.sync.dma_start(out=fblob[:], in_=fblob_ap[:])
            idx1_sb = iblob1[:]
            idx2_sb = iblob2[:, 0:NCH2 * 8]
            di0_sb = iblob2[:, NCH2 * 8:(NCH2 + DC2) * 8]
            di1_sb = iblob2[:, (NCH2 + DC2) * 8:]
            dstl1_sb = fblob[:, 0:NCH1]
            scl1_sb = fblob[:, NCH1:2 * NCH1]
            dstl2_sb = fblob[:, 2 * NCH1:2 * NCH1 + NCH2]
            dinv_sb = fblob[:, 2 * NCH1 + NCH2:2 * NCH1 + NCH2 + NT]
            dinv2_sb = fblob[:, 2 * NCH1 + NCH2 + NT:]

            w1f = pp.tile([H, H], F32)
            w2pf = pp.tile([H, 4], F32)
            nc.sync.dma_start(out=w1f[:], in_=w1_ap[:])
            nc.sync.dma_start(out=w2pf[:], in_=w2p_ap[:])
            w1b = pp.tile([H, H], BF16)
            w2pb = pp.tile([H, 4], BF16)
            nc.vector.tensor_copy(out=w1b[:], in_=w1f[:])
            nc.vector.tensor_copy(out=w2pb[:], in_=w2pf[:])

            iota_f = pp.tile([P, P], BF16)
            nc.gpsimd.iota(iota_f[:], pattern=[[1, P]], base=0,
                           channel_multiplier=0,
                           allow_small_or_imprecise_dtypes=True)

            y_sb = pp.tile([P, NT * 4], BF16)
            ydinv_sb = pp.tile([P, NT * 4], BF16)
            partA_sb = pp.tile([P, NT * 4], F32)
            qd_sb = pp.tile([P, NT * 4], F32)
            c_sb = pp.tile([P, NT * 4], BF16)
            out_sb = pp.tile([P, DC2 * 2], F32)

            SPLITS = [0, 13, 25, 37, NT]
            y_shard = dp.tile([PADN, 4], BF16)
            y_padA = dp.tile([NC * 1600, 128], BF16)
            y_padB = dp.tile([NC * 1536, 128], BF16)
            c_shard = dp.tile([PADN, 4], BF16)
            c_pad = dp.tile([NSLOT // 2, 128], BF16)
            y_fulls, c_fulls = [], []
            for k in range(4):
                nk = NC * (SPLITS[k + 1] - SPLITS[k]) * P
                y_fulls.append(dp.tile([nk, 4], BF16, name=f"y_full{k}"))
                c_fulls.append(dp.tile([nk, 4], BF16, name=f"c_full{k}"))

            xq2 = xq_ap[:].rearrange("(a b) e -> a b e", b=2)
            x_lo = xq2[:, 0, 0:H]
            x_hi = xq2[:, 1, 0:H]

            def build_oh(dstl_sb, c0, cnt, scl_sb=None, pool=None, mx=16,
                         act_nd=None):
                oh = (pool or ohp).tile([P, mx * P], BF16, tag="oh")
                for j in range(cnt):
                    if act_nd is not None and j % 4 == 0:
                        # ACT two-pass: a = |iota - d|; oh = relu(1 - a)
                        a = sp.tile([P, P], BF16, tag="oha")
                        nc.scalar.activation(
                            out=a[:], in_=iota_f[:],
                            func=mybir.ActivationFunctionType.Abs,
                            bias=act_nd[:, c0 + j:c0 + j + 1])
                        nc.scalar.activation(
                            out=oh[:, j * P:(j + 1) * P], in_=a[:],
                            func=mybir.ActivationFunctionType.Relu,
                            bias=1.0, scale=-1.0)
                        continue
                    nc.vector.tensor_scalar(
                        out=oh[:, j * P:(j + 1) * P],
                        in0=iota_f[:],
                        scalar1=dstl_sb[:, c0 + j:c0 + j + 1],
                        scalar2=(scl_sb[:, c0 + j:c0 + j + 1]
                                 if scl_sb is not None else None),
                        op0=mybir.AluOpType.is_equal,
                        op1=(mybir.AluOpType.mult
                             if scl_sb is not None else mybir.AluOpType.bypass),
                    )
                return oh[:, :cnt * P].rearrange("p (c e) -> p c e", e=P)

            # ---------------- Layer 1 + overlapped L2 pass-A ----------------
            Y_OFF_A = [0, 13 * 64]    # repad row offsets in y_padA (tiles 0-12, 13-24)
            Y_OFF_B = [0, 12 * 64]    # in y_padB (tiles 25-36, 37-48)

            def emit_y_split(k):
                t0, t1 = SPLITS[k], SPLITS[k + 1]
                nt = t1 - t0
                nc.sync.dma_start(
                    out=y_shard[t0 * P:t1 * P, :].rearrange(
                        "(t p) f -> p t f", p=P),
                    in_=y_sb[:, t0 * 4:t1 * 4].rearrange("p (t f) -> p t f", f=4),
                )
                if os.environ.get("K_NOCC"):
                    nc.sync.dma_start(out=y_fulls[k][0:P, :],
                                      in_=y_sb[:, t1 * 4 - 4:t1 * 4])
                else:
                    nc.gpsimd.collective_compute(
                        "AllGather", mybir.AluOpType.bypass,
                        replica_groups=[list(range(NC))],
                        ins=[y_shard[t0 * P:t1 * P, :].opt()],
                        outs=[y_fulls[k][:].opt()],
                    )
                pad, off = (y_padA, Y_OFF_A[k]) if k < 2 else (y_padB, Y_OFF_B[k - 2])
                nc.sync.dma_start(
                    out=pad[:, 0:8].rearrange(
                        "(c r) w -> c r w", c=NC)[:, off:off + nt * 64, :],
                    in_=y_fulls[k][:].rearrange(
                        "(c r q) f -> c r (q f)", c=NC, q=2),
                )

            gath2 = pp.tile([P, NCH2 * 4], BF16)
            g3d2 = gath2[:].rearrange("p (c e) -> p c e", e=4)

            def emit_l2_gathers(classes):
                for g in range(math.ceil(NT / G2)):
                    for k in classes:
                        base, cnt = s2["grp_runs"][g][k]
                        if cnt == 0:
                            continue
                        pad = y_padA if k < 2 else y_padB
                        _dma_gather_small(
                            nc.gpsimd,
                            out_ap=g3d2[:, base:base + cnt, :],
                            in_ap=pad[:, (k % 2) * 4:(k % 2) * 4 + 4],
                            idxs_ap=idx2_sb[:, base * 8:(base + cnt) * 8],
                            num_idxs=cnt * P,
                            elem_size=4,
                            elem_step=128,
                        )

            def emit_l2_tiles(t0, t1, classes, fold):
                for t in range(t0, t1):
                    runs = [(s2["chunk_start"][t][k], s2["nch_tc"][t][k])
                            for k in classes]
                    cnt = sum(cn for _, cn in runs)
                    if classes == (0, 1) and t in ohA_pre:
                        ohs = ohA_pre.pop(t)
                    else:
                        ohs = [build_oh(dstl2_sb, cb, cn, pool=ohp2, mx=mx2)
                               if cn else None for cb, cn in runs]
                    c_ps = psC.tile([P, 4], F32, tag="p4")
                    kk = 0
                    for (cb, cn), o3 in zip(runs, ohs):
                        for j in range(cn):
                            nc.tensor.matmul(
                                out=c_ps[:], lhsT=o3[:, j, :],
                                rhs=g3d2[:, cb + j, :],
                                start=(kk == 0), stop=(kk == cnt - 1),
                            )
                            kk += 1
                    fold(t, c_ps)

            def fold_a(t, c_ps):
                nc.scalar.copy(out=partA_sb[:, t * 4:t * 4 + 4], in_=c_ps[:])

            def fold_b(t, c_ps):
                nc.vector.scalar_tensor_tensor(
                    out=c_sb[:, t * 4:t * 4 + 4], in0=c_ps[:],
                    scalar=dinv_sb[:, t:t + 1],
                    in1=qd_sb[:, t * 4:t * 4 + 4],
                    op0=mybir.AluOpType.mult,
                    op1=mybir.AluOpType.add,
                )

            ohA_pre = {}

            def prebuild_passA(t0, t1):
                for t in range(t0, t1):
                    runs = [(s2["chunk_start"][t][k], s2["nch_tc"][t][k])
                            for k in (0, 1)]
                    ohA_pre[t] = [build_oh(dstl2_sb, cb, cn, pool=ohp2, mx=mx2)
                                  if cn else None for cb, cn in runs]

            ngrp1 = math.ceil(NT / G1)
            g1max = max(s1["grp_lo"][g][1] + s1["grp_hi"][g][1] for g in range(ngrp1))
            for g in range(ngrp1):
                if g == ngrp1 - 1:
                    nc.sync.dma_start(out=iblob2[:], in_=iblob2_ap[:])
                if phases >= 2 and g * G1 >= 28 and (g - 1) * G1 < 28:
                    prebuild_passA(0, 3)
                if phases >= 2 and g * G1 >= 40 and (g - 1) * G1 < 40:
                    prebuild_passA(3, 6)
                lo_b, lo_n = s1["grp_lo"][g]
                hi_b, hi_n = s1["grp_hi"][g]
                gcnt = lo_n + hi_n
                gq = g1p.tile([P, g1max * H], I8, tag="g1q")
                q3d = gq[:, :gcnt * H].rearrange("p (c e) -> p c e", e=H)
                for base, cnt, table in ((lo_b, lo_n, x_lo), (hi_b, hi_n, x_hi)):
                    if cnt == 0:
                        continue
                    _dma_gather_small(
                        nc.gpsimd,
                        out_ap=q3d[:, base - lo_b:base - lo_b + cnt, :],
                        in_ap=table,
                        idxs_ap=idx1_sb[:, base * 8:(base + cnt) * 8],
                        num_idxs=cnt * P,
                        elem_size=H,
                        elem_step=512,
                    )

                for t in range(g * G1, min((g + 1) * G1, NT)):
                    cl0, cn0 = s1["chunk_start"][t][0], s1["nch_tc"][t][0]
                    ch0, cn1 = s1["chunk_start"][t][1], s1["nch_tc"][t][1]
                    cnt = cn0 + cn1
                    msgs = sp.tile([P, 16 * H], BF16, tag="msgs")
                    nc.scalar.copy(out=msgs[:, :cn0 * H],
                                   in_=q3d[:, cl0 - lo_b:cl0 - lo_b + cn0, :])
                    if cn1:
                        nc.scalar.copy(out=msgs[:, cn0 * H:cnt * H],
                                       in_=q3d[:, ch0 - lo_b:ch0 - lo_b + cn1, :])
                    m3d = msgs[:, :cnt * H].rearrange("p (c e) -> p c e", e=H)
                    oh = build_oh(dstl1_sb, cl0, cn0, scl1_sb, mx=mx1)
                    oh2 = (build_oh(dstl1_sb, ch0, cn1, scl1_sb, mx=mx1)
                           if cn1 else None)
                    ut_ps = psA.tile([P, P], F32, tag="ut")
                    k = 0
                    for (cb, cn, o3) in ((cl0, cn0, oh), (ch0, cn1, oh2)):
                        for j in range(cn):
                            nc.tensor.matmul(
                                out=ut_ps[:], lhsT=m3d[:, k, :], rhs=o3[:, j, :],
                                start=(k == 0), stop=(k == cnt - 1),
                            )
                            k += 1
                    ut_sb = sp.tile([P, P], BF16, tag="utsb")
                    nc.scalar.copy(out=ut_sb[:], in_=ut_ps[:])
                    vt_ps = psB.tile([P, P], F32, tag="vt")
                    nc.tensor.matmul(out=vt_ps[:], lhsT=w1b[:], rhs=ut_sb[:],
                                     start=True, stop=True)
                    zt_sb = sp.tile([P, P], BF16, tag="ztsb")
                    nc.vector.tensor_relu(out=zt_sb[:], in_=vt_ps[:])
                    y_ps = psC.tile([P, 4], F32, tag="p4")
                    nc.tensor.matmul(out=y_ps[:], lhsT=zt_sb[:], rhs=w2pb[:],
                                     start=True, stop=True)
                    nc.vector.tensor_scalar(
                        out=y_sb[:, t * 4:t * 4 + 4], in0=y_ps[:],
                        scalar1=dinv2_sb[:, t:t + 1], scalar2=None,
                        op0=mybir.AluOpType.mult,
                    )

            for k in range(4):
                emit_y_split(k)
            if phases >= 2:
                emit_l2_gathers((0, 1))
                emit_l2_tiles(0, NT, (0, 1), fold_a)

            # self-loop term: ydinv = y * dinv; qd = (passA + ydinv) * dinv
            nc.vector.tensor_tensor(
                out=ydinv_sb[:].rearrange("p (t f) -> p t f", f=4),
                in0=y_sb[:].rearrange("p (t f) -> p t f", f=4),
                in1=dinv_sb[:].to_broadcast([P, NT, 4]),
                op=mybir.AluOpType.mult,
            )

            def emit_c_split(k):
                t0, t1 = SPLITS[k], SPLITS[k + 1]
                nc.sync.dma_start(
                    out=c_shard[t0 * P:t1 * P, :].rearrange(
                        "(t p) f -> p t f", p=P),
                    in_=c_sb[:, t0 * 4:t1 * 4].rearrange(
                        "p (t f) -> p t f", f=4),
                )
                if os.environ.get("K_NOCC"):
                    nc.sync.dma_start(out=c_fulls[k][0:P, :],
                                      in_=c_sb[:, t1 * 4 - 4:t1 * 4])
                else:
                    nc.gpsimd.collective_compute(
                        "AllGather", mybir.AluOpType.bypass,
                        replica_groups=[list(range(NC))],
                        ins=[c_shard[t0 * P:t1 * P, :].opt()],
                        outs=[c_fulls[k][:].opt()],
                    )
                nc.sync.dma_start(
                    out=c_pad[:, 0:8].rearrange(
                        "(c r) w -> c r w", c=NC)[:, SPLITS[k] * 64:SPLITS[k + 1] * 64, :],
                    in_=c_fulls[k][:].rearrange(
                        "(c r q) f -> c r (q f)", c=NC, q=2),
                )

            # ---------------- Layer 2 pass-B ----------------
            if phases >= 2:
                emit_l2_gathers((2, 3))
                nc.vector.tensor_tensor(
                    out=qd_sb[:].rearrange("p (t f) -> p t f", f=4),
                    in0=partA_sb[:].rearrange("p (t f) -> p t f", f=4),
                    in1=dinv_sb[:].to_broadcast([P, NT, 4]),
                    op=mybir.AluOpType.mult,
                )
                nc.vector.tensor_tensor(
                    out=qd_sb[:], in0=qd_sb[:], in1=ydinv_sb[:],
                    op=mybir.AluOpType.add,
                )
                for k in range(4):
                    emit_l2_tiles(SPLITS[k], SPLITS[k + 1], (2, 3), fold_b)
                    emit_c_split(k)


            # ---------------- Decode ----------------
            if phases >= 3:
                g0 = g2p.tile([P, DC2 * 4], BF16, tag="dg0")
                g1_ = g2p.tile([P, DC2 * 4], BF16, tag="dg1")
                # e0 parity runs: buckets (0,0)+(0,1) even, (1,0)+(1,1) odd
                e0_runs = [(dbase[0], dcb[0] + dcb[1], 0), (dbase[2], dcb[2] + dcb[3], 1)]
                e1_runs = [(dbase[b], dcb[b], b % 2) for b in range(4)]
                for gt, di, runs in ((g0, di0_sb, e0_runs), (g1_, di1_sb, e1_runs)):
                    gv = gt[:].rearrange("p (c e) -> p c e", e=4)
                    for base, cnt, par in runs:
                        if cnt == 0:
                            continue
                        _dma_gather_small(
                            nc.gpsimd,
                            out_ap=gv[:, base:base + cnt, :],
                            in_ap=c_pad[:, par * 4:par * 4 + 4],
                            idxs_ap=di[:, base * 8:(base + cnt) * 8],
                            num_idxs=cnt * P,
                            elem_size=4,
                            elem_step=128,
                        )
                nc.vector.tensor_tensor(
                    out=out_sb[:].rearrange("p (c e) -> p c e", e=2),
                    in0=g0[:].rearrange("p (c e) -> p c e", e=4)[:, :, 0:2],
                    in1=g1_[:].rearrange("p (c e) -> p c e", e=4)[:, :, 2:4],
                    op=mybir.AluOpType.add,
                )
            else:
                nc.vector.memset(out_sb[:], 0)

            nc.sync.dma_start(out=out_ap[:], in_=out_sb[:])

    nc.compile()
    return nc


def kernel(x, edge_index, pos_edge_index, neg_edge_index, W1, W2, Wlin):
    x = np.asarray(x, np.float32)
    W1 = np.asarray(W1, np.float32)
    W2 = np.asarray(W2, np.float32)
    Wlin = np.asarray(Wlin, np.float32)

    per_core, shape = _preprocess(x, edge_index, pos_edge_index, neg_edge_index)

    # fold W2 and Wlin: cols 0,1 pair with e0 (Wlin[:, :128]), cols 2,3 with e1
    Wl = np.stack([Wlin[0, :H], Wlin[1, :H], Wlin[0, H:], Wlin[1, H:]], axis=1)
    W2p = (W2 @ Wl).astype(np.float32)

    # xq = int8 row-quantized dinv * x (scale rides in the one-hot)
    ei = np.asarray(edge_index)
    dst = np.concatenate([ei[1], np.arange(N)]).astype(np.int64)
    deg = np.bincount(dst, minlength=N).astype(np.float64)
    dinv = 1.0 / np.sqrt(deg)
    xd = x * dinv[:, None].astype(np.float32)
    s_node = np.abs(xd).max(axis=1) / 127.0
    xq = np.zeros((N, 256), np.int8)
    xq[:, 0:H] = np.round(xd / s_node[:, None]).astype(np.int8)

    nc = _build_program(shape)

    s1, s2 = shape["l1"], shape["l2"]
    NCH1, NCH2, DC2 = s1["nch"], s2["nch"], shape["dc2"]
    in_maps = []
    for c in range(NC):
        m = per_core[c]
        iblob1 = m["idx1"]
        iblob2 = np.concatenate([m["idx2"], m["d_idx0"], m["d_idx1"]], axis=1)
        fblob = np.concatenate(
            [m["dstl1"], m["scl1"], m["dstl2"], m["dinv_t"], m["dinv2_t"]], axis=1
        ).astype(np.float32)
        in_maps.append(dict(xq=xq, w1=W1, w2p=W2p, iblob1=iblob1,
                            iblob2=iblob2, fblob=fblob))

    res = run_bass_kernel_spmd(nc, in_maps, core_ids=list(range(NC)))

    out = np.empty((NE_EVAL, 2), np.float32)
    for c in range(NC):
        shard = res.results[c]["out"]  # [P, DC2*2]; slot (p, k) at [p, 2k:2k+2]
        order, slot = per_core[c]["unperm"]
        vals = shard.reshape(P, DC2, 2)[slot % P, slot // P]
        out[c * EV_PC + order] = vals
    return out


# revision 57
# speedup vs baseline: 1.0653x; 1.0101x over previous
"""Trainium2 Bass kernel for a 2-layer GCN link predictor (NetLinkTrain).

Math: z = relu(A @ (x @ W1)); z2 = A @ (z @ W2); out = [z2[e0], z2[e1]] @ Wlin.T
where A = D^-1/2 (Adj + I) D^-1/2.

Key algebraic factorizations:
  * W2/Wlin folding: W2' = W2 @ [Wlin[:, :128].T | Wlin[:, 128:].T] ([128, 4]),
    so layer 2 and the decode work on 4-wide node features.
  * The symmetric norm dinv[src]*dinv[dst] factors entirely out of both
    aggregations: gather from xd = dinv*x, then
      z = dinv[dst] * relu(W1^T @ sum xd[src])        (relu commutes, dinv>0)
      y = dinv[dst]^2 * (z_hat @ W2')                 (folds both dst factors)
      c = dinv[dst] * sum y[src]
    so the scatter one-hots are pure (iota == dst) compares with no weights.

Data movement (the memory-bound core of the problem):
  * L1 gathers 256B bf16 xd rows per edge (dma_gather, 1 desc/edge).
  * L2/decode gather ONLY the 4 bf16 values needed per edge (8B elements at a
    256B-aligned element stride from a padded table, 2 nodes per 256B row).
    Sub-256B elements need a patched dma_gather builder (the stock bass
    assert requires elem_size%256B; the hardware only requires the element
    STRIDE to be a 256B multiple -- verified on device).
  * Aggregation per 128-dst tile: TensorE scatter matmul with a DVE-built
    one-hot, accumulating in PSUM.

Sharding: edges are sharded by destination-node range (core c owns nodes
[c*6250, (c+1)*6250)); each core owns its segment sums completely, so the only
collectives are two small AllGathers (y and c, ~50-100KB per core).

Host (numpy) does index preprocessing only: self-loops, degrees, sorting edges
by (dst tile, table class), wrapped int16 index layout, and the xd cast.
"""

import math
import os
import sys

import numpy as np

sys.path.insert(0, "/opt/trn_rl_repo")

import concourse.bacc as bacc
import concourse.bass as bass
import concourse.tile as tile
from concourse import ap_utils, mybir
from concourse.bass_utils import run_bass_kernel_spmd

N = 50000
H = 128
P = 128
NC = 8
NPC = N // NC              # 6250 nodes per core
NT = math.ceil(NPC / P)    # 49 dst tiles per core
PADN = NT * P              # 6272 padded nodes per core
NSLOT = NC * PADN          # 50176 packed slots
LO = 25000                 # x-table half split (int16 index limit)
G1 = 4                     # L1 tiles per gather call group
G2 = 13                    # L2 tiles per gather call group
NE_EVAL = 200000
EV_PC = NE_EVAL // NC      # 25000 eval edges per core

F32 = mybir.dt.float32
BF16 = mybir.dt.bfloat16
I16 = mybir.dt.int16
I8 = mybir.dt.int8

import ml_dtypes

BF16_NP = ml_dtypes.bfloat16


def _dma_gather_small(g, out_ap, in_ap, idxs_ap, num_idxs, elem_size, elem_step,
                      queue_num=0):
    """bass.BassGpSimd.dma_gather (DRAM source, non-transpose) without the
    elem_size%256B assert. The element STRIDE (elem_step bytes) must still be
    a 256B multiple; sub-256B elem_size verified on hardware."""
    g._assert_queue_num(queue_num)
    assert idxs_ap.dtype == mybir.dt.int16
    assert in_ap.dtype == out_ap.dtype
    assert ap_utils.ap_is_contiguous(in_ap.ap[1:])
    assert ap_utils.ap_is_contiguous(out_ap.ap[1:])
    assert ap_utils.ap_is_contiguous(idxs_ap.ap[1:])
    assert in_ap.ap[-1][1] == out_ap.ap[-1][1] == elem_size
    assert in_ap.ap[0][0] == elem_step
    stride_bytes = elem_step * mybir.dt.size(in_ap.dtype)
    stride_bytes_256, rem = divmod(stride_bytes, 256)
    assert rem == 0 and stride_bytes_256 < 256
    _in_ap = g.lower_ap_dma(in_ap, for_custom_bir_dma=True)
    return g.add_instruction(
        mybir.InstDMAGatherAnt(
            name=g.bass.get_next_instruction_name(),
            ins=[*_in_ap, g.lower_ap(idxs_ap),
                 g.lower_val_access(g.to_reg(num_idxs))],
            outs=[g.lower_ap(out_ap)],
            transpose=False,
            num_idxs=num_idxs,
            elem_size=elem_size,
            stride_bytes_256=stride_bytes_256,
            gen_mode=0,
            single_packet=False,
            queue_num=queue_num,
            sbuf_tokens_per_rank=0,
            sbuf_free_dim_per_rank=0,
            sbuf_free_dim_pad_per_rank=0,
            sbuf_byte_offset=0,
        )
    )


def _packed_id(n):
    """Packed slot of node n in the AllGathered y/c buffers: core-major,
    then natural node order (tile-major)."""
    n = np.asarray(n)
    c = n // NPC
    return c * PADN + (n - c * NPC)


def _wrap_idx(v, n_chunks):
    """v: [n_chunks, 128] int -> dma_gather wrapped idx layout [128, n_chunks*8]."""
    a16 = v.reshape(n_chunks, 8, 16).transpose(2, 0, 1).reshape(16, n_chunks * 8)
    return np.tile(a16, (8, 1)).astype(np.int16)


def _edge_layout(core, tl, cls, dloc, idxval, group_sz, sval=None, ncls=2):
    """Lay out edges into (tile, class)-grouped 128-slot chunks.

    Gather-call contiguity: chunks ordered group-major; within a group all
    class-k chunks (tile-major) for k = 0..ncls-1.

    Returns (idx_wrapped, dstl[, scale]) per core plus shared shape info.
    """
    key = (core * NT + tl) * ncls + cls
    counts = np.zeros((NC, NT, ncls), np.int64)
    np.add.at(counts, (core, tl, cls), 1)
    nch_tc = -(-counts.max(axis=0) // P)  # [NT, ncls] chunks per (tile, class)

    ngrp = math.ceil(NT / group_sz)
    chunk_start = np.zeros((NT, ncls), np.int64)
    grp_runs = [[] for _ in range(ngrp)]  # per group: [(base, count)] per class
    pos = 0
    for g in range(ngrp):
        t0, t1 = g * group_sz, min((g + 1) * group_sz, NT)
        for k in range(ncls):
            base = pos
            for t in range(t0, t1):
                chunk_start[t, k] = pos
                pos += nch_tc[t, k]
            grp_runs[g].append((base, pos - base))
    grp_lo = [r[0] for r in grp_runs]
    grp_hi = [r[1] for r in grp_runs]
    nch = pos

    order = np.argsort(key, kind="stable")
    s_key = key[order]
    group_start = np.concatenate([[0], np.cumsum(counts.reshape(-1))])[:-1]
    rank = np.arange(len(order)) - group_start[s_key]
    s_core = s_key // (NT * ncls)
    s_t = (s_key // ncls) % NT
    s_cls = s_key % ncls
    dest = chunk_start[s_t, s_cls] * P + rank

    per_core = []
    for c in range(NC):
        m = s_core == c
        slot_idx = np.zeros(nch * P, np.int64)
        slot_dstl = np.full(nch * P, 255.0, np.float32)
        d = dest[m]
        slot_idx[d] = idxval[order][m]
        slot_dstl[d] = dloc[order][m]
        ent = [_wrap_idx(slot_idx.reshape(nch, P), nch),
               slot_dstl.reshape(nch, P).T.copy()]
        if sval is not None:
            slot_s = np.zeros(nch * P, np.float32)
            slot_s[d] = sval[order][m]
            ent.append(slot_s.reshape(nch, P).T.copy())
        per_core.append(tuple(ent))
    shape = dict(nch=nch, nch_tc=nch_tc.tolist(), chunk_start=chunk_start.tolist(),
                 grp_lo=grp_lo, grp_hi=grp_hi, grp_runs=grp_runs)
    return per_core, shape


def _preprocess(x, edge_index, pos_edge_index, neg_edge_index):
    x = np.asarray(x, np.float32)
    ei = np.asarray(edge_index)
    src = np.concatenate([ei[0], np.arange(N)]).astype(np.int64)
    dst = np.concatenate([ei[1], np.arange(N)]).astype(np.int64)
    deg = np.bincount(dst, minlength=N).astype(np.float64)
    dinv = 1.0 / np.sqrt(deg)  # every node has a self loop -> deg >= 1

    core = dst // NPC
    dl = dst - core * NPC
    tl = dl // P
    dloc = (dl % P).astype(np.float32)

    # L1: class = src parity (even/odd 256B rows of the int8 x table);
    # per-slot int8 dequant scale rides in the one-hot's second op
    xd = x.astype(np.float32) * dinv[:, None].astype(np.float32)
    s_node = (np.abs(xd).max(axis=1) / 127.0).astype(np.float32)
    l1, shape1 = _edge_layout(core, tl, src % 2, dloc, src // 2, G1,
                              sval=s_node[src])
    # L2: 4 classes = (src tile-half, slot parity); half-A sources (local
    # tiles 0-24) live in a separate padded table that is complete mid-L1,
    # so half-A aggregation overlaps the end of layer 1. The appended
    # self-loops are excluded -- their contribution is added from the local
    # y shard on-device (exact, no gather).
    ne_real = ei.shape[1]
    score = src // NPC
    soff = src - score * NPC
    in_b = (soff >= 3200).astype(np.int64)
    row = np.where(in_b == 0, score * 1600 + soff // 2,
                   score * 1536 + (soff - 3200) // 2)
    cls4 = in_b * 2 + soff % 2
    l2, shape2 = _edge_layout(core[:ne_real], tl[:ne_real], cls4[:ne_real],
                              dloc[:ne_real], row[:ne_real], G2, ncls=4)

    # per-(p, t) dinv and dinv^2 for the post-aggregation scales
    offs = np.arange(NT * P).reshape(NT, P)  # off = t*128 + p
    dinv_t = np.zeros((NC, P, NT), np.float32)
    dinv2_t = np.zeros((NC, P, NT), np.float32)
    for c in range(NC):
        v = np.zeros(NT * P)
        vv = dinv[c * NPC:(c + 1) * NPC]
        v[:NPC] = vv
        dinv_t[c] = v[offs].T
        v[:NPC] = vv * vv
        dinv2_t[c] = v[offs].T

    # decode: bucket eval edges by (slot(e0)%2, slot(e1)%2)
    e0 = np.concatenate([np.asarray(pos_edge_index[0]), np.asarray(neg_edge_index[0])])
    e1 = np.concatenate([np.asarray(pos_edge_index[1]), np.asarray(neg_edge_index[1])])
    s0, s1 = _packed_id(e0), _packed_id(e1)
    bkt = (s0 % 2) * 2 + (s1 % 2)
    bcnt = np.zeros((NC, 4), np.int64)
    for c in range(NC):
        bcnt[c] = np.bincount(bkt[c * EV_PC:(c + 1) * EV_PC], minlength=4)
    dcb = (-(-bcnt.max(axis=0) // P)).tolist()  # chunks per bucket (shared)
    dbase = np.concatenate([[0], np.cumsum(dcb)]).tolist()
    dc2 = int(dbase[-1])

    dec = []
    for c in range(NC):
        sl = slice(c * EV_PC, (c + 1) * EV_PC)
        b = bkt[sl]
        order = np.argsort(b, kind="stable")
        rank = np.arange(EV_PC) - np.concatenate([[0], np.cumsum(bcnt[c])])[:-1][b[order]]
        slot = np.array(dbase)[b[order]] * P + rank
        i0 = np.zeros(dc2 * P, np.int64)
        i1 = np.zeros(dc2 * P, np.int64)
        i0[slot] = (s0[sl] // 2)[order]
        i1[slot] = (s1[sl] // 2)[order]
        dec.append(dict(
            d_idx0=_wrap_idx(i0.reshape(dc2, P), dc2),
            d_idx1=_wrap_idx(i1.reshape(dc2, P), dc2),
            # slot -> position in this core's eval range
            unperm=(np.asarray(order), slot),
        ))

    per_core = []
    for c in range(NC):
        per_core.append(dict(
            idx1=l1[c][0], dstl1=l1[c][1], scl1=l1[c][2],
            idx2=l2[c][0], dstl2=l2[c][1],
            dinv_t=dinv_t[c], dinv2_t=dinv2_t[c],
            d_idx0=dec[c]["d_idx0"], d_idx1=dec[c]["d_idx1"],
            unperm=dec[c]["unperm"],
        ))
    shape = dict(l1=shape1, l2=shape2, dcb=dcb, dbase=dbase, dc2=dc2)
    return per_core, shape


def _build_program(shape):
    s1, s2 = shape["l1"], shape["l2"]
    NCH1, NCH2, DC2 = s1["nch"], s2["nch"], shape["dc2"]
    dbase, dcb = shape["dbase"], shape["dcb"]
    mx1 = max(max(r) for r in s1["nch_tc"])
    mx2 = max(max(r) for r in s2["nch_tc"])
    shape_l1_grp = [
        (s1["grp_lo"][g][0], s1["grp_hi"][g][0] + s1["grp_hi"][g][1])
        for g in range(len(s1["grp_lo"]))
    ]

    nc = bacc.Bacc("TRN2", target_bir_lowering=False, debug=False, num_devices=NC)

    xq_ap = nc.dram_tensor("xq", [N, 256], I8, kind="ExternalInput").ap()
    w1_ap = nc.dram_tensor("w1", [H, H], F32, kind="ExternalInput").ap()
    w2p_ap = nc.dram_tensor("w2p", [H, 4], F32, kind="ExternalInput").ap()
    # metadata blobs: one int16, one f32
    IW1 = NCH1 * 8
    IW2 = (NCH2 + 2 * DC2) * 8
    FW = 2 * NCH1 + NCH2 + 2 * NT
    iblob1_ap = nc.dram_tensor("iblob1", [P, IW1], I16, kind="ExternalInput").ap()
    iblob2_ap = nc.dram_tensor("iblob2", [P, IW2], I16, kind="ExternalInput").ap()
    fblob_ap = nc.dram_tensor("fblob", [P, FW], F32, kind="ExternalInput").ap()
    out_ap = nc.dram_tensor("out", [P, DC2 * 2], F32, kind="ExternalOutput").ap()

    phases = int(os.environ.get("K_PHASES", "3"))

    with tile.TileContext(nc) as tc:
        with (
            tc.tile_pool(name="persist", bufs=1) as pp,
            tc.tile_pool(name="g1", bufs=6) as g1p,
            tc.tile_pool(name="g2", bufs=3) as g2p,
            tc.tile_pool(name="oh", bufs=12) as ohp,
            tc.tile_pool(name="oh2", bufs=40) as ohp2,
            tc.tile_pool(name="small", bufs=8) as sp,
            tc.tile_pool(name="psA", bufs=4, space="PSUM") as psA,
            tc.tile_pool(name="psB", bufs=2, space="PSUM") as psB,
            tc.tile_pool(name="psC", bufs=2, space="PSUM") as psC,
            tc.tile_pool(name="dram", bufs=1, space="DRAM") as dp,
        ):
            # ---- persistent metadata ----
            iblob1 = pp.tile([P, IW1], I16)
            iblob2 = pp.tile([P, IW2], I16)
            fblob = pp.tile([P, FW], F32)
            for _g in range(math.ceil(NT / G1)):
                _lo = shape_l1_grp[_g][0] * 8
                _hi = shape_l1_grp[_g][1] * 8
                nc.sync.dma_start(out=iblob1[:, _lo:_hi], in_=iblob1_ap[:, _lo:_hi])
                if _g == 0:
                    nc.sync.dma_start(out=fblob[:], in_=fblob_ap[:])
            idx1_sb = iblob1[:]
            idx2_sb = iblob2[:, 0:NCH2 * 8]
            di0_sb = iblob2[:, NCH2 * 8:(NCH2 + DC2) * 8]
            di1_sb = iblob2[:, (NCH2 + DC2) * 8:]
            dstl1_sb = fblob[:, 0:NCH1]
            scl1_sb = fblob[:, NCH1:2 * NCH1]
            dstl2_sb = fblob[:, 2 * NCH1:2 * NCH1 + NCH2]
            dinv_sb = fblob[:, 2 * NCH1 + NCH2:2 * NCH1 + NCH2 + NT]
            dinv2_sb = fblob[:, 2 * NCH1 + NCH2 + NT:]

            w1f = pp.tile([H, H], F32)
            w2pf = pp.tile([H, 4], F32)
            nc.sync.dma_start(out=w1f[:], in_=w1_ap[:])
            nc.sync.dma_start(out=w2pf[:], in_=w2p_ap[:])
            w1b = pp.tile([H, H], BF16)
            w2pb = pp.tile([H, 4], BF16)
            nc.vector.tensor_copy(out=w1b[:], in_=w1f[:])
            nc.vector.tensor_copy(out=w2pb[:], in_=w2pf[:])

            iota_f = pp.tile([P, P], BF16)
            nc.gpsimd.iota(iota_f[:], pattern=[[1, P]], base=0,
                           channel_multiplier=0,
                           allow_small_or_imprecise_dtypes=True)

            y_sb = pp.tile([P, NT * 4], BF16)
            ydinv_sb = pp.tile([P, NT * 4], BF16)
            partA_sb = pp.tile([P, NT * 4], F32)
            qd_sb = pp.tile([P, NT * 4], F32)
            c_sb = pp.tile([P, NT * 4], BF16)
            out_sb = pp.tile([P, DC2 * 2], F32)

            SPLITS = [0, 13, 25, 37, NT]
            y_shard = dp.tile([PADN, 4], BF16)
            y_padA = dp.tile([NC * 1600, 128], BF16)
            y_padB = dp.tile([NC * 1536, 128], BF16)
            c_shard = dp.tile([PADN, 4], BF16)
            c_pad = dp.tile([NSLOT // 2, 128], BF16)
            y_fulls, c_fulls = [], []
            for k in range(4):
                nk = NC * (SPLITS[k + 1] - SPLITS[k]) * P
                y_fulls.append(dp.tile([nk, 4], BF16, name=f"y_full{k}"))
                c_fulls.append(dp.tile([nk, 4], BF16, name=f"c_full{k}"))

            xq2 = xq_ap[:].rearrange("(a b) e -> a b e", b=2)
            x_lo = xq2[:, 0, 0:H]
            x_hi = xq2[:, 1, 0:H]

            def build_oh(dstl_sb, c0, cnt, scl_sb=None, pool=None, mx=16,
                         act_nd=None):
                oh = (pool or ohp).tile([P, mx * P], BF16, tag="oh")
                for j in range(cnt):
                    if act_nd is not None and j % 4 == 0:
                        # ACT two-pass: a = |iota - d|; oh = relu(1 - a)
                        a = sp.tile([P, P], BF16, tag="oha")
                        nc.scalar.activation(
                            out=a[:], in_=iota_f[:],
                            func=mybir.ActivationFunctionType.Abs,
                            bias=act_nd[:, c0 + j:c0 + j + 1])
                        nc.scalar.activation(
                            out=oh[:, j * P:(j + 1) * P], in_=a[:],
                            func=mybir.ActivationFunctionType.Relu,
                            bias=1.0, scale=-1.0)
                        continue
                    nc.vector.tensor_scalar(
                        out=oh[:, j * P:(j + 1) * P],
                        in0=iota_f[:],
                        scalar1=dstl_sb[:, c0 + j:c0 + j + 1],
                        scalar2=(scl_sb[:, c0 + j:c0 + j + 1]
                                 if scl_sb is not None else None),
                        op0=mybir.AluOpType.is_equal,
                        op1=(mybir.AluOpType.mult
                             if scl_sb is not None else mybir.AluOpType.bypass),
                    )
                return oh[:, :cnt * P].rearrange("p (c e) -> p c e", e=P)

            # ---------------- Layer 1 + overlapped L2 pass-A ----------------
            Y_OFF_A = [0, 13 * 64]    # repad row offsets in y_padA (tiles 0-12, 13-24)
            Y_OFF_B = [0, 12 * 64]    # in y_padB (tiles 25-36, 37-48)

            def emit_y_split(k):
                t0, t1 = SPLITS[k], SPLITS[k + 1]
                nt = t1 - t0
                nc.sync.dma_start(
                    out=y_shard[t0 * P:t1 * P, :].rearrange(
                        "(t p) f -> p t f", p=P),
                    in_=y_sb[:, t0 * 4:t1 * 4].rearrange("p (t f) -> p t f", f=4),
                )
                if os.environ.get("K_NOCC"):
                    nc.sync.dma_start(out=y_fulls[k][0:P, :],
                                      in_=y_sb[:, t1 * 4 - 4:t1 * 4])
                else:
                    nc.gpsimd.collective_compute(
                        "AllGather", mybir.AluOpType.bypass,
                        replica_groups=[list(range(NC))],
                        ins=[y_shard[t0 * P:t1 * P, :].opt()],
                        outs=[y_fulls[k][:].opt()],
                    )
                pad, off = (y_padA, Y_OFF_A[k]) if k < 2 else (y_padB, Y_OFF_B[k - 2])
                nc.sync.dma_start(
                    out=pad[:, 0:8].rearrange(
                        "(c r) w -> c r w", c=NC)[:, off:off + nt * 64, :],
                    in_=y_fulls[k][:].rearrange(
                        "(c r q) f -> c r (q f)", c=NC, q=2),
                )

            gath2 = pp.tile([P, NCH2 * 4], BF16)
            g3d2 = gath2[:].rearrange("p (c e) -> p c e", e=4)

            def emit_l2_gathers(classes):
                for g in range(math.ceil(NT / G2)):
                    for k in classes:
                        base, cnt = s2["grp_runs"][g][k]
                        if cnt == 0:
                            continue
                        pad = y_padA if k < 2 else y_padB
                        _dma_gather_small(
                            nc.gpsimd,
                            out_ap=g3d2[:, base:base + cnt, :],
                            in_ap=pad[:, (k % 2) * 4:(k % 2) * 4 + 4],
                            idxs_ap=idx2_sb[:, base * 8:(base + cnt) * 8],
                            num_idxs=cnt * P,
                            elem_size=4,
                            elem_step=128,
                        )

            def emit_l2_tiles(t0, t1, classes, fold):
                for t in range(t0, t1):
                    runs = [(s2["chunk_start"][t][k], s2["nch_tc"][t][k])
                            for k in classes]
                    cnt = sum(cn for _, cn in runs)
                    if classes == (0, 1) and t in ohA_pre:
                        ohs = ohA_pre.pop(t)
                    else:
                        ohs = [build_oh(dstl2_sb, cb, cn, pool=ohp2, mx=mx2)
                               if cn else None for cb, cn in runs]
                    c_ps = psC.tile([P, 4], F32, tag="p4")
                    kk = 0
                    for (cb, cn), o3 in zip(runs, ohs):
                        for j in range(cn):
                            nc.tensor.matmul(
                                out=c_ps[:], lhsT=o3[:, j, :],
                                rhs=g3d2[:, cb + j, :],
                                start=(kk == 0), stop=(kk == cnt - 1),
                            )
                            kk += 1
                    fold(t, c_ps)

            def fold_a(t, c_ps):
                nc.scalar.copy(out=partA_sb[:, t * 4:t * 4 + 4], in_=c_ps[:])

            def fold_b(t, c_ps):
                nc.vector.scalar_tensor_tensor(
                    out=c_sb[:, t * 4:t * 4 + 4], in0=c_ps[:],
                    scalar=dinv_sb[:, t:t + 1],
                    in1=qd_sb[:, t * 4:t * 4 + 4],
                    op0=mybir.AluOpType.mult,
                    op1=mybir.AluOpType.add,
                )

            ohA_pre = {}

            def prebuild_passA(t0, t1):
                for t in range(t0, t1):
                    runs = [(s2["chunk_start"][t][k], s2["nch_tc"][t][k])
                            for k in (0, 1)]
                    ohA_pre[t] = [build_oh(dstl2_sb, cb, cn, pool=ohp2, mx=mx2)
                                  if cn else None for cb, cn in runs]

            ngrp1 = math.ceil(NT / G1)
            g1max = max(s1["grp_lo"][g][1] + s1["grp_hi"][g][1] for g in range(ngrp1))
            for g in range(ngrp1):
                if g == ngrp1 - 1:
                    nc.sync.dma_start(out=iblob2[:], in_=iblob2_ap[:])
                if phases >= 2 and g * G1 >= 28 and (g - 1) * G1 < 28:
                    prebuild_passA(0, 3)
                if phases >= 2 and g * G1 >= 40 and (g - 1) * G1 < 40:
                    prebuild_passA(3, 6)
                lo_b, lo_n = s1["grp_lo"][g]
                hi_b, hi_n = s1["grp_hi"][g]
                gcnt = lo_n + hi_n
                gq = g1p.tile([P, g1max * H], I8, tag="g1q")
                q3d = gq[:, :gcnt * H].rearrange("p (c e) -> p c e", e=H)
                for base, cnt, table in ((lo_b, lo_n, x_lo), (hi_b, hi_n, x_hi)):
                    if cnt == 0:
                        continue
                    _dma_gather_small(
                        nc.gpsimd,
                        out_ap=q3d[:, base - lo_b:base - lo_b + cnt, :],
                        in_ap=table,
                        idxs_ap=idx1_sb[:, base * 8:(base + cnt) * 8],
                        num_idxs=cnt * P,
                        elem_size=H,
                        elem_step=512,
                    )

                for t in range(g * G1, min((g + 1) * G1, NT)):
                    cl0, cn0 = s1["chunk_start"][t][0], s1["nch_tc"][t][0]
                    ch0, cn1 = s1["chunk_start"][t][1], s1["nch_tc"][t][1]
                    cnt = cn0 + cn1
                    msgs = sp.tile([P, 16 * H], BF16, tag="msgs")
                    nc.scalar.copy(out=msgs[:, :cn0 * H],
                                   in_=q3d[:, cl0 - lo_b:cl0 - lo_b + cn0, :])
                    if cn1:
                        nc.scalar.copy(out=msgs[:, cn0 * H:cnt * H],
                                       in_=q3d[:, ch0 - lo_b:ch0 - lo_b + cn1, :])
                    m3d = msgs[:, :cnt * H].rearrange("p (c e) -> p c e", e=H)
                    oh = build_oh(dstl1_sb, cl0, cn0, scl1_sb, mx=mx1)
                    oh2 = (build_oh(dstl1_sb, ch0, cn1, scl1_sb, mx=mx1)
                           if cn1 else None)
                    ut_ps = psA.tile([P, P], F32, tag="ut")
                    k = 0
                    for (cb, cn, o3) in ((cl0, cn0, oh), (ch0, cn1, oh2)):
                        for j in range(cn):
                            nc.tensor.matmul(
                                out=ut_ps[:], lhsT=m3d[:, k, :], rhs=o3[:, j, :],
                                start=(k == 0), stop=(k == cnt - 1),
                            )
                            k += 1
                    ut_sb = sp.tile([P, P], BF16, tag="utsb")
                    nc.scalar.copy(out=ut_sb[:], in_=ut_ps[:])
                    vt_ps = psB.tile([P, P], F32, tag="vt")
                    nc.tensor.matmul(out=vt_ps[:], lhsT=w1b[:], rhs=ut_sb[:],
                                     start=True, stop=True)
                    zt_sb = sp.tile([P, P], BF16, tag="ztsb")
                    nc.vector.tensor_relu(out=zt_sb[:], in_=vt_ps[:])
                    y_ps = psC.tile([P, 4], F32, tag="p4")
                    nc.tensor.matmul(out=y_ps[:], lhsT=zt_sb[:], rhs=w2pb[:],
                                     start=True, stop=True)
                    nc.vector.tensor_scalar(
                        out=y_sb[:, t * 4:t * 4 + 4], in0=y_ps[:],
                        scalar1=dinv2_sb[:, t:t + 1], scalar2=None,
                        op0=mybir.AluOpType.mult,
                    )

            for k in range(4):
                emit_y_split(k)
            if phases >= 2:
                emit_l2_gathers((0, 1))
                emit_l2_tiles(0, NT, (0, 1), fold_a)

            # self-loop term: ydinv = y * dinv; qd = (passA + ydinv) * dinv
            nc.vector.tensor_tensor(
                out=ydinv_sb[:].rearrange("p (t f) -> p t f", f=4),
                in0=y_sb[:].rearrange("p (t f) -> p t f", f=4),
                in1=dinv_sb[:].to_broadcast([P, NT, 4]),
                op=mybir.AluOpType.mult,
            )

            def emit_c_split(k):
                t0, t1 = SPLITS[k], SPLITS[k + 1]
                nc.sync.dma_start(
                    out=c_shard[t0 * P:t1 * P, :].rearrange(
                        "(t p) f -> p t f", p=P),
                    in_=c_sb[:, t0 * 4:t1 * 4].rearrange(
                        "p (t f) -> p t f", f=4),
                )
                if os.environ.get("K_NOCC"):
                    nc.sync.dma_start(out=c_fulls[k][0:P, :],
                                      in_=c_sb[:, t1 * 4 - 4:t1 * 4])
                else:
                    nc.gpsimd.collective_compute(
                        "AllGather", mybir.AluOpType.bypass,
                        replica_groups=[list(range(NC))],
                        ins=[c_shard[t0 * P:t1 * P, :].opt()],
                        outs=[c_fulls[k][:].opt()],
                    )
                nc.sync.dma_start(
                    out=c_pad[:, 0:8].rearrange(
                        "(c r) w -> c r w", c=NC)[:, SPLITS[k] * 64:SPLITS[k + 1] * 64, :],
                    in_=c_fulls[k][:].rearrange(
                        "(c r q) f -> c r (q f)", c=NC, q=2),
                )

            # ---------------- Layer 2 pass-B ----------------
            if phases >= 2:
                emit_l2_gathers((2, 3))
                nc.vector.tensor_tensor(
                    out=qd_sb[:].rearrange("p (t f) -> p t f", f=4),
                    in0=partA_sb[:].rearrange("p (t f) -> p t f", f=4),
                    in1=dinv_sb[:].to_broadcast([P, NT, 4]),
                    op=mybir.AluOpType.mult,
                )
                nc.vector.tensor_tensor(
                    out=qd_sb[:], in0=qd_sb[:], in1=ydinv_sb[:],
                    op=mybir.AluOpType.add,
                )
                for k in range(4):
                    emit_l2_tiles(SPLITS[k], SPLITS[k + 1], (2, 3), fold_b)
                    emit_c_split(k)


            # ---------------- Decode ----------------
            if phases >= 3:
                g0 = g2p.tile([P, DC2 * 4], BF16, tag="dg0")
                g1_ = g2p.tile([P, DC2 * 4], BF16, tag="dg1")
                # e0 parity runs: buckets (0,0)+(0,1) even, (1,0)+(1,1) odd
                e0_runs = [(dbase[0], dcb[0] + dcb[1], 0), (dbase[2], dcb[2] + dcb[3], 1)]
                e1_runs = [(dbase[b], dcb[b], b % 2) for b in range(4)]
                for gt, di, runs in ((g0, di0_sb, e0_runs), (g1_, di1_sb, e1_runs)):
                    gv = gt[:].rearrange("p (c e) -> p c e", e=4)
                    for base, cnt, par in runs:
                        if cnt == 0:
                            continue
                        _dma_gather_small(
                            nc.gpsimd,
                            out_ap=gv[:, base:base + cnt, :],
                            in_ap=c_pad[:, par * 4:par * 4 + 4],
                            idxs_ap=di[:, base * 8:(base + cnt) * 8],
                            num_idxs=cnt * P,
                            elem_size=4,
                            elem_step=128,
                        )
                nc.vector.tensor_tensor(
                    out=out_sb[:].rearrange("p (c e) -> p c e", e=2),
                    in0=g0[:].rearrange("p (c e) -> p c e", e=4)[:, :, 0:2],
                    in1=g1_[:].rearrange("p (c e) -> p c e", e=4)[:, :, 2:4],
                    op=mybir.AluOpType.add,
                )
            else:
                nc.vector.memset(out_sb[:], 0)

            nc.sync.dma_start(out=out_ap[:], in_=out_sb[:])

    nc.compile()
    return nc


def kernel(x, edge_index, pos_edge_index, neg_edge_index, W1, W2, Wlin):
    x = np.asarray(x, np.float32)
    W1 = np.asarray(W1, np.float32)
    W2 = np.asarray(W2, np.float32)
    Wlin = np.asarray(Wlin, np.float32)

    per_core, shape = _preprocess(x, edge_index, pos_edge_index, neg_edge_index)

    # fold W2 and Wlin: cols 0,1 pair with e0 (Wlin[:, :128]), cols 2,3 with e1
    Wl = np.stack([Wlin[0, :H], Wlin[1, :H], Wlin[0, H:], Wlin[1, H:]], axis=1)
    W2p = (W2 @ Wl).astype(np.float32)

    # xq = int8 row-quantized dinv * x (scale rides in the one-hot)
    ei = np.asarray(edge_index)
    dst = np.concatenate([ei[1], np.arange(N)]).astype(np.int64)
    deg = np.bincount(dst, minlength=N).astype(np.float64)
    dinv = 1.0 / np.sqrt(deg)
    xd = x * dinv[:, None].astype(np.float32)
    s_node = np.abs(xd).max(axis=1) / 127.0
    xq = np.zeros((N, 256), np.int8)
    xq[:, 0:H] = np.round(xd / s_node[:, None]).astype(np.int8)

    nc = _build_program(shape)

    s1, s2 = shape["l1"], shape["l2"]
    NCH1, NCH2, DC2 = s1["nch"], s2["nch"], shape["dc2"]
    in_maps = []
    for c in range(NC):
        m = per_core[c]
        iblob1 = m["idx1"]
        iblob2 = np.concatenate([m["idx2"], m["d_idx0"], m["d_idx1"]], axis=1)
        fblob = np.concatenate(
            [m["dstl1"], m["scl1"], m["dstl2"], m["dinv_t"], m["dinv2_t"]], axis=1
        ).astype(np.float32)
        in_maps.append(dict(xq=xq, w1=W1, w2p=W2p, iblob1=iblob1,
                            iblob2=iblob2, fblob=fblob))

    res = run_bass_kernel_spmd(nc, in_maps, core_ids=list(range(NC)))

    out = np.empty((NE_EVAL, 2), np.float32)
    for c in range(NC):
        shard = res.results[c]["out"]  # [P, DC2*2]; slot (p, k) at [p, 2k:2k+2]
        order, slot = per_core[c]["unperm"]
        vals = shard.reshape(P, DC2, 2)[slot % P, slot // P]
        out[c * EV_PC + order] = vals
    return out
